# revision 8
# baseline (speedup 1.0000x reference)
"""Trainium2 Bass kernel for nn_EnhancedLossModule (contrastive + triplet +
focal + label-smoothing loss over B=2048, C=1000, D=512).

Strategy (8 NeuronCores, SPMD):
  - Shard rows of the [B,B] gram/distance matrices: 256 rows per core.
  - One fp32 matmul per core G = f_local @ f_all^T ([256,2048]) feeds both the
    contrastive similarity (sim = G*s_i*s_j) and squared distances
    (d^2 = r_i + r_j - 2G).
  - Triplet loss: sum over same-label (anchor, positive) pairs of
    sum_n relu(d_ap + margin - d_an) * [label_n != label_a].  The mask is
    folded into the dense distance matrix (same-label entries pushed to a
    huge value so the relu kills them).  Self-pairs (a==p) read the dense
    rows directly; real pairs gather their anchor rows via dma_gather.
  - Focal + label smoothing: data-parallel over the pred shard.
  - Each core DMAs out a [128, NCOL] tile of per-partition partial sums;
    the host sums them (the scalar "all-reduce") and combines the losses.
"""

import math

import numpy as np

import concourse.bacc as bacc
import concourse.bass as bass
import concourse.tile as tile
from concourse import mybir
from concourse.bass_utils import run_bass_kernel_spmd

# ---- problem constants (hardcoded per the task spec) ----
B, C, D = 2048, 1000, 512
N_CORES = 8
R = B // N_CORES          # rows per core = 256
RT = R // 128             # row tiles per core = 2
KT = D // 128             # contraction tiles = 4
NCHUNK = 4                # 2048 / 512 psum chunks

TEMPERATURE = 0.07
C_MARGIN = 0.5
T_MARGIN = 1.0
GAMMA = 2.0
ALPHA = 0.25
SMOOTHING = 0.1
W_CONTRASTIVE = 0.1
W_TRIPLET = 0.1
W_FOCAL = 0.4
W_LABEL_SMOOTH = 0.4

BIGMASK = 1.0e9           # added to d^2 of same-label entries
OFF = SMOOTHING / (C - 1)

F32 = mybir.dt.float32
ALU = mybir.AluOpType
AF = mybir.ActivationFunctionType

_BUILD_CACHE: dict = {}


def _build(nt_p: int):
    """Build + compile the SPMD bass program for nt_p pair tiles per core."""
    key = nt_p
    if key in _BUILD_CACHE:
        return _BUILD_CACHE[key]

    KP = nt_p * 128

    # accumulator column map
    COL_NEG = 0                    # 2 cols: sum min(sim-0.5, 0), per row tile
    COL_SELF = 2                   # 2 cols: sum min(D'-margin, 0), per row tile
    COL_PAIR = 4                   # nt_p cols: valid*sum min(D'row-x, 0)
    COL_POS = COL_PAIR + nt_p      # 1 col: sum valid * -ln(exp(sim/T)+eps)
    COL_NCO = COL_POS + 1          # 1 col: sum valid * min(simp, 0.5)
    COL_FOC = COL_NCO + 1          # 2 cols: sum (1-pt)^2*ce, per row tile
    COL_LS = COL_FOC + 2           # 2 cols: label-smoothing per row tile
    NCOL = COL_LS + 2

    nc = bacc.Bacc(
        "TRN2", target_bir_lowering=False, debug=False, num_devices=N_CORES
    )

    # ---- DRAM I/O ----
    featT = nc.dram_tensor("featT", [D, B], F32, kind="ExternalInput")
    featN = nc.dram_tensor("featN", [B, D], F32, kind="ExternalInput")
    featTl = nc.dram_tensor("featTl", [D, R], F32, kind="ExternalInput")
    featl = nc.dram_tensor("featl", [R, D], F32, kind="ExternalInput")
    predl = nc.dram_tensor("predl", [R, C], F32, kind="ExternalInput")
    lab_all = nc.dram_tensor("lab_all", [B], F32, kind="ExternalInput")
    lab_loc = nc.dram_tensor("lab_loc", [R, 1], F32, kind="ExternalInput")
    tgt_loc = nc.dram_tensor("tgt_loc", [R, 1], F32, kind="ExternalInput")
    pfi = nc.dram_tensor("pfi", [KP, D], F32, kind="ExternalInput")
    pfp = nc.dram_tensor("pfp", [KP, D], F32, kind="ExternalInput")
    pidx = nc.dram_tensor("pidx", [128, nt_p * 8], mybir.dt.int16,
                          kind="ExternalInput")
    pvalid = nc.dram_tensor("pvalid", [128, nt_p], F32, kind="ExternalInput")
    acc_out = nc.dram_tensor("acc_out", [128, NCOL], F32, kind="ExternalOutput")

    def bcast_ap(handle, n):
        a = handle.ap() if isinstance(handle, bass.DRamTensorHandle) \
            else handle[:, :]
        return bass.AP(tensor=a.tensor, offset=a.offset, ap=[[0, 128], [1, n]])

    with tile.TileContext(nc) as tc:
        with (
            tc.tile_pool(name="persist", bufs=1) as persist,
            tc.tile_pool(name="stream", bufs=3) as stream,
            tc.tile_pool(name="dense", bufs=2) as dense,
            tc.tile_pool(name="small", bufs=2) as small,
            tc.tile_pool(name="gpsum", bufs=2, space="PSUM") as gpsum,
            tc.tile_pool(name="dscratch", bufs=1, space="DRAM") as dscratch,
        ):
            dma = nc.sync

            # DRAM scratch as pool tiles so Tile tracks store->read deps
            dp_dram = dscratch.tile([R, B], F32, tag="dp")
            rrow_dram = dscratch.tile([16, 128], F32, tag="rrow")
            srow_dram = dscratch.tile([16, 128], F32, tag="srow")

            # ---------- constants / setup ----------
            zeros = persist.tile([128, B], F32)
            nc.gpsimd.memset(zeros, 0.0)
            acc = persist.tile([128, NCOL], F32)
            nc.vector.memset(acc, 0.0)
            iota_c = persist.tile([128, C], F32)
            nc.gpsimd.iota(iota_c, pattern=[[1, C]], base=0,
                           channel_multiplier=0,
                           allow_small_or_imprecise_dtypes=True)
            # identity for PE transpose
            iota_sq = persist.tile([128, 128], F32)
            nc.gpsimd.iota(iota_sq, pattern=[[1, 128]], base=0,
                           channel_multiplier=0,
                           allow_small_or_imprecise_dtypes=True)
            pid = persist.tile([128, 1], F32)
            nc.gpsimd.iota(pid, pattern=[[0, 1]], base=0,
                           channel_multiplier=1,
                           allow_small_or_imprecise_dtypes=True)
            ident = persist.tile([128, 128], F32)
            nc.vector.tensor_scalar(out=ident, in0=iota_sq, scalar1=pid,
                                    scalar2=None, op0=ALU.is_equal)

            # ---------- big persistent loads ----------
            ft = []          # featT tiles [128, B] x KT
            for k in range(KT):
                t = persist.tile([128, B], F32, tag=f"ft{k}")
                dma.dma_start(out=t, in_=featT.ap()[k * 128:(k + 1) * 128, :])
                ft.append(t)
            ftl = []         # featT local tiles [128, R] x KT
            for k in range(KT):
                t = persist.tile([128, R], F32, tag=f"ftl{k}")
                dma.dma_start(out=t, in_=featTl.ap()[k * 128:(k + 1) * 128, :])
                ftl.append(t)

            lab_b = persist.tile([128, B], F32)
            nc.gpsimd.dma_start(out=lab_b, in_=bcast_ap(lab_all, B))

            # ---------- r_all via square+accum over natural feature tiles ----
            rcols = persist.tile([128, 16], F32)
            for t_i in range(16):
                fn_t = stream.tile([128, D], F32, tag="fn")
                dma.dma_start(out=fn_t,
                              in_=featN.ap()[t_i * 128:(t_i + 1) * 128, :])
                if t_i % 2 == 0:
                    nc.scalar.activation(out=fn_t, in_=fn_t, func=AF.Square,
                                         accum_out=rcols[:, t_i:t_i + 1])
                else:
                    nc.vector.scalar_tensor_tensor(
                        out=fn_t, in0=fn_t, scalar=1.0, in1=fn_t,
                        op0=ALU.mult, op1=ALU.mult,
                        accum_out=rcols[:, t_i:t_i + 1])

            rho_cols = persist.tile([128, 16], F32)
            nc.scalar.activation(out=rho_cols, in_=rcols, func=AF.Sqrt)
            s_cols = persist.tile([128, 16], F32)
            nc.vector.reciprocal(out=s_cols, in_=rho_cols)

            # transpose rcols/s_cols -> [16,128] rows -> DRAM -> broadcast
            for src, dst_dram in ((rcols, rrow_dram), (s_cols, srow_dram)):
                ps = gpsum.tile([16, 128], F32, tag="gpsum_big")
                nc.tensor.transpose(ps, src, ident)
                row_sb = small.tile([16, 128], F32, tag="rowsb")
                nc.scalar.copy(out=row_sb, in_=ps)
                dma.dma_start(out=dst_dram[:], in_=row_sb)

            r_b = persist.tile([128, B], F32)
            nc.gpsimd.dma_start(out=r_b, in_=bcast_ap(rrow_dram, B))
            s_b = persist.tile([128, B], F32)
            nc.gpsimd.dma_start(out=s_b, in_=bcast_ap(srow_dram, B))

            # ---------- local row stats ----------
            rloc = persist.tile([128, RT], F32)
            for m in range(RT):
                fl_t = stream.tile([128, D], F32, tag="fn")
                dma.dma_start(out=fl_t,
                              in_=featl.ap()[m * 128:(m + 1) * 128, :])
                nc.scalar.activation(out=fl_t, in_=fl_t, func=AF.Square,
                                     accum_out=rloc[:, m:m + 1])
            rho_loc = persist.tile([128, RT], F32)
            nc.scalar.activation(out=rho_loc, in_=rloc, func=AF.Sqrt)
            s_loc = persist.tile([128, RT], F32)
            nc.vector.reciprocal(out=s_loc, in_=rho_loc)

            lab_l = persist.tile([128, RT], F32)
            dma.dma_start(out=lab_l[:, 0:1], in_=lab_loc.ap()[0:128, :])
            dma.dma_start(out=lab_l[:, 1:2], in_=lab_loc.ap()[128:256, :])

            # ---------- dense phase: G matmul + sim/dist ----------
            for m in range(RT):
                gps = gpsum.tile([128, B], F32, tag="gpsum_big")
                for nchunk in range(NCHUNK):
                    for k in range(KT):
                        nc.tensor.matmul(
                            gps[:, nchunk * 512:(nchunk + 1) * 512],
                            ftl[k][:, m * 128:(m + 1) * 128],
                            ft[k][:, nchunk * 512:(nchunk + 1) * 512],
                            start=(k == 0), stop=(k == KT - 1),
                        )
                # lmb = (lab_b == lab_i) * BIGMASK
                lmb = dense.tile([128, B], F32, tag="dwork")
                nc.vector.tensor_scalar(out=lmb, in0=lab_b,
                                        scalar1=lab_l[:, m:m + 1],
                                        scalar2=BIGMASK,
                                        op0=ALU.is_equal, op1=ALU.mult)
                # rbl = lmb + r_i + r_j
                rbl = dense.tile([128, B], F32, tag="dwork")
                nc.vector.scalar_tensor_tensor(
                    out=rbl, in0=lmb, scalar=rloc[:, m:m + 1], in1=r_b,
                    op0=ALU.add, op1=ALU.add)
                # d2m = -2*G + rbl
                d2m = dense.tile([128, B], F32, tag="dwork")
                nc.vector.scalar_tensor_tensor(
                    out=d2m, in0=gps, scalar=-2.0, in1=rbl,
                    op0=ALU.mult, op1=ALU.add)
                # sim = (G * s_i) * s_j   -- also accumulate neg term
                sim = dense.tile([128, B], F32, tag="dwork")
                nc.vector.scalar_tensor_tensor(
                    out=sim, in0=gps, scalar=s_loc[:, m:m + 1], in1=s_b,
                    op0=ALU.mult, op1=ALU.mult)
                nc.vector.scalar_tensor_tensor(
                    out=sim, in0=sim, scalar=0.5, in1=zeros,
                    op0=ALU.subtract, op1=ALU.min,
                    accum_out=acc[:, COL_NEG + m:COL_NEG + m + 1])
                # D' = sqrt(relu(d2m))
                d2r = dense.tile([128, B], F32, tag="dwork")
                nc.scalar.activation(out=d2r, in_=d2m, func=AF.Relu)
                dpt = dense.tile([128, B], F32, tag="dwork")
                nc.scalar.activation(out=dpt, in_=d2r, func=AF.Sqrt)
                dma.dma_start(out=dp_dram[m * 128:(m + 1) * 128, :],
                              in_=dpt)
                # self-pair triplet term: sum min(D' - margin, 0)
                nc.vector.scalar_tensor_tensor(
                    out=dpt, in0=dpt, scalar=float(T_MARGIN), in1=zeros,
                    op0=ALU.subtract, op1=ALU.min,
                    accum_out=acc[:, COL_SELF + m:COL_SELF + m + 1])

            # ---------- pair smalls: ri, rp, G over pair features ----------
            pk = persist.tile([128, 3 * nt_p], F32)   # [d2 | ri | rp] columns
            pk_g = persist.tile([128, nt_p], F32)
            for g in range(nt_p):
                fi_t = stream.tile([128, D], F32, tag="pfi")
                dma.dma_start(out=fi_t,
                              in_=pfi.ap()[g * 128:(g + 1) * 128, :])
                fp_t = stream.tile([128, D], F32, tag="pfp")
                dma.dma_start(out=fp_t,
                              in_=pfp.ap()[g * 128:(g + 1) * 128, :])
                scr_c = stream.tile([128, D], F32, tag="pscr")
                nc.vector.scalar_tensor_tensor(
                    out=scr_c, in0=fi_t, scalar=1.0, in1=fp_t,
                    op0=ALU.mult, op1=ALU.mult,
                    accum_out=pk_g[:, g:g + 1])
                nc.vector.scalar_tensor_tensor(
                    out=fi_t, in0=fi_t, scalar=1.0, in1=fi_t,
                    op0=ALU.mult, op1=ALU.mult,
                    accum_out=pk[:, nt_p + g:nt_p + g + 1])
                nc.vector.scalar_tensor_tensor(
                    out=fp_t, in0=fp_t, scalar=1.0, in1=fp_t,
                    op0=ALU.mult, op1=ALU.mult,
                    accum_out=pk[:, 2 * nt_p + g:2 * nt_p + g + 1])

            # d2 = ri + rp - 2*G  (into pk[:, 0:nt_p])
            t_neg2g = small.tile([128, nt_p], F32, tag="pneg2g")
            nc.vector.scalar_tensor_tensor(
                out=t_neg2g, in0=pk_g, scalar=-2.0,
                in1=pk[:, nt_p:2 * nt_p], op0=ALU.mult, op1=ALU.add)
            nc.vector.tensor_add(pk[:, 0:nt_p], t_neg2g,
                                 pk[:, 2 * nt_p:3 * nt_p])
            # roots of [d2 | ri | rp] in one ACT pass
            proots = persist.tile([128, 3 * nt_p], F32)
            nc.scalar.activation(out=proots, in_=pk, func=AF.Sqrt)
            # x = d + margin
            px = persist.tile([128, nt_p], F32)
            nc.vector.tensor_scalar(out=px, in0=proots[:, 0:nt_p],
                                    scalar1=float(T_MARGIN), scalar2=None,
                                    op0=ALU.add)
            # 1/sqrt(ri), 1/sqrt(rp)
            pinv = small.tile([128, 2 * nt_p], F32, tag="pinv")
            nc.vector.reciprocal(out=pinv, in_=proots[:, nt_p:3 * nt_p])
            # simp = G * inv_i * inv_p
            simp = small.tile([128, nt_p], F32, tag="simp")
            nc.vector.tensor_mul(simp, pk_g, pinv[:, 0:nt_p])
            nc.vector.tensor_mul(simp, simp, pinv[:, nt_p:2 * nt_p])

            pval_t = persist.tile([128, nt_p], F32)
            dma.dma_start(out=pval_t, in_=pvalid.ap())

            # pos pair loss: -ln(exp(simp/T) + 1e-8), masked, accumulated
            pexp = small.tile([128, nt_p], F32, tag="pexp")
            nc.scalar.activation(out=pexp, in_=simp, func=AF.Exp,
                                 scale=float(1.0 / TEMPERATURE))
            pexp1 = small.tile([128, nt_p], F32, tag="pexp1")
            nc.vector.tensor_scalar(out=pexp1, in0=pexp, scalar1=1e-8,
                                    scalar2=None, op0=ALU.add)
            pln = small.tile([128, nt_p], F32, tag="pln")
            nc.scalar.activation(out=pln, in_=pexp1, func=AF.Ln)
            nc.vector.scalar_tensor_tensor(
                out=pln, in0=pln, scalar=-1.0, in1=pval_t,
                op0=ALU.mult, op1=ALU.mult,
                accum_out=acc[:, COL_POS:COL_POS + 1])
            # neg correction: min(simp, 0.5) * valid
            pmc = small.tile([128, nt_p], F32, tag="pmc")
            nc.vector.tensor_scalar(out=pmc, in0=simp, scalar1=0.5,
                                    scalar2=None, op0=ALU.min)
            nc.vector.scalar_tensor_tensor(
                out=pmc, in0=pmc, scalar=1.0, in1=pval_t,
                op0=ALU.mult, op1=ALU.mult,
                accum_out=acc[:, COL_NCO:COL_NCO + 1])

            # ---------- pair row gather + triplet accumulation ----------
            idx_sb = persist.tile([128, nt_p * 8], mybir.dt.int16)
            dma.dma_start(out=idx_sb, in_=pidx.ap())
            for g in range(nt_p):
                grow = stream.tile([128, 1, B], F32, tag="grow")
                nc.gpsimd.dma_gather(
                    out_ap=grow,
                    in_ap=dp_dram[:, :],
                    idxs_ap=idx_sb[:, g * 8:(g + 1) * 8],
                    num_idxs=128,
                    num_idxs_reg=128,
                    elem_size=B,
                )
                tmp_t = small.tile([128, 1], F32, tag="ptrip")
                nc.vector.scalar_tensor_tensor(
                    out=grow[:, 0, :], in0=grow[:, 0, :], scalar=px[:, g:g + 1],
                    in1=zeros, op0=ALU.subtract, op1=ALU.min,
                    accum_out=tmp_t)
                nc.vector.tensor_mul(
                    acc[:, COL_PAIR + g:COL_PAIR + g + 1], tmp_t,
                    pval_t[:, g:g + 1])

            # ---------- focal + label smoothing ----------
            for m in range(RT):
                pred_t = stream.tile([128, C], F32, tag="pred")
                dma.dma_start(out=pred_t,
                              in_=predl.ap()[m * 128:(m + 1) * 128, :])
                tgt_t = small.tile([128, 1], F32, tag="tgt")
                dma.dma_start(out=tgt_t,
                              in_=tgt_loc.ap()[m * 128:(m + 1) * 128, :])
                # sum(exp(pred)) -> lse (no max-subtraction; pred ~ randn)
                escr = stream.tile([128, C], F32, tag="escr")
                se = small.tile([128, 1], F32, tag="se")
                nc.scalar.activation(out=escr, in_=pred_t, func=AF.Exp,
                                     accum_out=se)
                lse = small.tile([128, 1], F32, tag="lse")
                nc.scalar.activation(out=lse, in_=se, func=AF.Ln)
                # sum(pred)
                spred = small.tile([128, 1], F32, tag="spred")
                nc.vector.scalar_tensor_tensor(
                    out=escr, in0=pred_t, scalar=1.0, in1=zeros[:, 0:C],
                    op0=ALU.mult, op1=ALU.add, accum_out=spred)
                # pred[target]
                tmask = stream.tile([128, C], F32, tag="tmask")
                nc.vector.tensor_scalar(out=tmask, in0=iota_c, scalar1=tgt_t,
                                        scalar2=None, op0=ALU.is_equal)
                ptgt = small.tile([128, 1], F32, tag="ptgt")
                nc.vector.scalar_tensor_tensor(
                    out=escr, in0=pred_t, scalar=1.0, in1=tmask,
                    op0=ALU.mult, op1=ALU.mult, accum_out=ptgt)
                # ce = lse - ptgt ; pt = exp(-ce)
                ce = small.tile([128, 1], F32, tag="ce")
                nc.vector.tensor_sub(ce, lse, ptgt)
                pt = small.tile([128, 1], F32, tag="pt")
                nc.scalar.activation(out=pt, in_=ce, func=AF.Exp, scale=-1.0)
                onept = small.tile([128, 1], F32, tag="onept")
                nc.vector.tensor_scalar(out=onept, in0=pt, scalar1=-1.0,
                                        scalar2=1.0, op0=ALU.mult, op1=ALU.add)
                f2 = small.tile([128, 1], F32, tag="f2")
                nc.vector.tensor_mul(f2, onept, onept)
                nc.vector.tensor_mul(
                    acc[:, COL_FOC + m:COL_FOC + m + 1], f2, ce)
                # ls_i = lse - OFF*spred - (0.9-OFF)*ptgt
                t1 = small.tile([128, 1], F32, tag="lst1")
                nc.vector.tensor_scalar(out=t1, in0=spred,
                                        scalar1=float(-OFF), scalar2=None,
                                        op0=ALU.mult)
                t2 = small.tile([128, 1], F32, tag="lst2")
                nc.vector.scalar_tensor_tensor(
                    out=t2, in0=ptgt,
                    scalar=float(-(1.0 - SMOOTHING - OFF)), in1=t1,
                    op0=ALU.mult, op1=ALU.add)
                nc.vector.tensor_add(
                    acc[:, COL_LS + m:COL_LS + m + 1], lse, t2)

            # ---------- writeback ----------
            dma.dma_start(out=acc_out.ap(), in_=acc)

    nc.compile()
    meta = dict(nt_p=nt_p, NCOL=NCOL, COL_NEG=COL_NEG, COL_SELF=COL_SELF,
                COL_PAIR=COL_PAIR, COL_POS=COL_POS, COL_NCO=COL_NCO,
                COL_FOC=COL_FOC, COL_LS=COL_LS)
    _BUILD_CACHE[key] = (nc, meta)
    return nc, meta


def _host_prep(pred, target, features):
    """Build the 8 per-core input maps."""
    pred = np.asarray(pred, dtype=np.float32)
    target = np.asarray(target)
    features = np.asarray(features, dtype=np.float32)
    labels = target.astype(np.int64)

    featT = np.ascontiguousarray(features.T)               # [D, B]
    lab_f32 = labels.astype(np.float32)

    # same-label non-self pairs, grouped by anchor's core
    order = np.argsort(labels, kind="stable")
    sorted_lab = labels[order]
    # boundaries of equal-label runs
    starts = np.flatnonzero(np.r_[True, sorted_lab[1:] != sorted_lab[:-1]])
    ends = np.r_[starts[1:], len(sorted_lab)]
    pairs_i, pairs_p = [], []
    for s, e in zip(starts, ends):
        if e - s < 2:
            continue
        members = order[s:e]
        ii, pp = np.meshgrid(members, members, indexing="ij")
        m = ii != pp
        pairs_i.append(ii[m])
        pairs_p.append(pp[m])
    if pairs_i:
        pairs_i = np.concatenate(pairs_i)
        pairs_p = np.concatenate(pairs_p)
    else:
        pairs_i = np.zeros((0,), np.int64)
        pairs_p = np.zeros((0,), np.int64)
    k_real = len(pairs_i)

    core_of = pairs_i // R
    per_core = [(pairs_i[core_of == c], pairs_p[core_of == c])
                for c in range(N_CORES)]
    kmax = max((len(a) for a, _ in per_core), default=0)
    nt_p = max(1, math.ceil(kmax / 128))
    KP = nt_p * 128

    e1 = np.zeros((D,), np.float32)
    e1[0] = 1.0

    in_maps = []
    for c in range(N_CORES):
        pi, pp = per_core[c]
        k = len(pi)
        fi = np.empty((KP, D), np.float32)
        fp = np.empty((KP, D), np.float32)
        fi[:k] = features[pi]
        fp[:k] = features[pp]
        fi[k:] = e1
        fp[k:] = e1
        rowidx = np.zeros((KP,), np.int16)
        rowidx[:k] = (pi % R).astype(np.int16)
        valid = np.zeros((KP,), np.float32)
        valid[:k] = 1.0
        # gather idx layout: unwrapped[j] = idx_tile[j%16, j//16], per tile g
        # [p, g*8+s] = rowidx[g*128 + s*16 + p]; replicated into all 8
        # GPSIMD core windows (HW reads its own 16-partition group)
        idx16 = rowidx.reshape(nt_p, 8, 16).transpose(2, 0, 1).reshape(16, -1)
        pidx = np.ascontiguousarray(np.tile(idx16, (8, 1)))
        # valid layout: [128, nt_p] with [p, g] = valid[g*128+p]
        pval = np.ascontiguousarray(valid.reshape(nt_p, 128).T)

        rows = slice(c * R, (c + 1) * R)
        in_maps.append({
            "featT": featT,
            "featN": features,
            "featTl": np.ascontiguousarray(featT[:, rows]),
            "featl": np.ascontiguousarray(features[rows]),
            "predl": np.ascontiguousarray(pred[rows]),
            "lab_all": lab_f32,
            "lab_loc": np.ascontiguousarray(lab_f32[rows, None]),
            "tgt_loc": np.ascontiguousarray(lab_f32[rows, None]),
            "pfi": fi,
            "pfp": fp,
            "pidx": pidx,
            "pvalid": pval,
        })
    return in_maps, nt_p, k_real


def _combine(results, meta, k_real):
    """Host-side scalar all-reduce + final loss combination."""
    nt_p = meta["nt_p"]
    accs = np.stack([r["acc_out"] for r in results]).astype(np.float64)
    tot = accs.sum(axis=(0, 1))                 # [NCOL]

    neg_dense = -(tot[meta["COL_NEG"]] + tot[meta["COL_NEG"] + 1])
    self_trip = -(tot[meta["COL_SELF"]] + tot[meta["COL_SELF"] + 1])
    pair_trip = -tot[meta["COL_PAIR"]:meta["COL_PAIR"] + nt_p].sum()
    pair_pos = tot[meta["COL_POS"]]
    negcorr_min = tot[meta["COL_NCO"]]
    focal_sum = tot[meta["COL_FOC"]] + tot[meta["COL_FOC"] + 1]
    ls_sum = tot[meta["COL_LS"]] + tot[meta["COL_LS"] + 1]

    k_tot = k_real + B
    pos_self = B * (-np.log(np.exp(1.0 / TEMPERATURE) + 1e-8))
    pos_zero = (B * B - k_tot) * (-np.log1p(1e-8))
    pos_sum = pair_pos + pos_self + pos_zero
    neg_sum = neg_dense + negcorr_min + 0.5 * B

    lc = (pos_sum + neg_sum) / (B * B)
    lt = (self_trip + pair_trip) / (B + 1e-8)
    lf = ALPHA * focal_sum / B
    ls = ls_sum / B
    total = (W_CONTRASTIVE * lc + W_TRIPLET * lt
             + W_FOCAL * lf + W_LABEL_SMOOTH * ls)
    return np.array([lc, lt, lf, ls, total], dtype=np.float32)


def kernel(pred, target, features):
    in_maps, nt_p, k_real = _host_prep(pred, target, features)
    # fix the tgt entries (they must be the class targets, same as labels here)
    nc, meta = _build(nt_p)
    res = run_bass_kernel_spmd(nc, in_maps, core_ids=list(range(N_CORES)))
    return _combine(res.results, meta, k_real)


if __name__ == "__main__":
    import reference

    inputs = reference.setup_inputs()
    expected = np.asarray(reference.reference(**inputs))
    actual = kernel(**{k: np.asarray(v) for k, v in inputs.items()})
    err = np.abs(actual - expected) / np.maximum(np.abs(expected), 1e-12)
    print("expected:", expected)
    print("actual:  ", actual)
    print("rel err: ", err)


# revision 11
# speedup vs baseline: 1.1013x; 1.1013x over previous
"""Trainium2 Bass kernel for nn_EnhancedLossModule (contrastive + triplet +
focal + label-smoothing loss over B=2048, C=1000, D=512).

Strategy (8 NeuronCores, SPMD):
  - Shard rows of the [B,B] gram/distance matrices: 256 rows per core.
  - One fp32 matmul per core G = f_local @ f_all^T ([256,2048]) feeds both the
    contrastive similarity (sim = G*s_i*s_j) and squared distances
    (d^2 = r_i + r_j - 2G).
  - Triplet loss: sum over same-label (anchor, positive) pairs of
    sum_n relu(d_ap + margin - d_an) * [label_n != label_a].  The mask is
    folded into the dense distance matrix (same-label entries pushed to a
    huge value so the relu kills them).  Self-pairs (a==p) read the dense
    rows directly; real pairs gather their anchor rows via dma_gather.
  - Focal + label smoothing: data-parallel over the pred shard.
  - Each core DMAs out a [128, NCOL] tile of per-partition partial sums;
    the host sums them (the scalar "all-reduce") and combines the losses.
"""

import math

import numpy as np

import concourse.bacc as bacc
import concourse.bass as bass
import concourse.tile as tile
from concourse import mybir
from concourse.bass_utils import run_bass_kernel_spmd

# ---- problem constants (hardcoded per the task spec) ----
B, C, D = 2048, 1000, 512
N_CORES = 8
R = B // N_CORES          # rows per core = 256
RT = R // 128             # row tiles per core = 2
KT = D // 128             # contraction tiles = 4
NCHUNK = 4                # 2048 / 512 psum chunks

TEMPERATURE = 0.07
C_MARGIN = 0.5
T_MARGIN = 1.0
GAMMA = 2.0
ALPHA = 0.25
SMOOTHING = 0.1
W_CONTRASTIVE = 0.1
W_TRIPLET = 0.1
W_FOCAL = 0.4
W_LABEL_SMOOTH = 0.4

BIGMASK = 1.0e9           # added to d^2 of same-label entries
OFF = SMOOTHING / (C - 1)

F32 = mybir.dt.float32
ALU = mybir.AluOpType
AF = mybir.ActivationFunctionType

_BUILD_CACHE: dict = {}


def _build(nt_p: int):
    """Build + compile the SPMD bass program for nt_p pair tiles per core."""
    key = nt_p
    if key in _BUILD_CACHE:
        return _BUILD_CACHE[key]

    KP = nt_p * 128

    # accumulator column map
    COL_NEG = 0                    # 2 cols: sum min(sim-0.5, 0), per row tile
    COL_SELF = 2                   # 2 cols: sum min(D'-margin, 0), per row tile
    COL_PAIR = 4                   # nt_p cols: valid*sum min(D'row-x, 0)
    COL_POS = COL_PAIR + nt_p      # 1 col: sum valid * -ln(exp(sim/T)+eps)
    COL_NCO = COL_POS + 1          # 1 col: sum valid * min(simp, 0.5)
    COL_FOC = COL_NCO + 1          # 2 cols: sum (1-pt)^2*ce, per row tile
    COL_LS = COL_FOC + 2           # 2 cols: label-smoothing per row tile
    NCOL = COL_LS + 2

    nc = bacc.Bacc(
        "TRN2", target_bir_lowering=False, debug=False, num_devices=N_CORES
    )

    # ---- DRAM I/O ----
    featT = nc.dram_tensor("featT", [D, B], mybir.dt.float32r,
                       kind="ExternalInput")
    featN = nc.dram_tensor("featN", [B, D], F32, kind="ExternalInput")
    featTl = nc.dram_tensor("featTl", [D, R], mybir.dt.float32r,
                        kind="ExternalInput")
    featl = nc.dram_tensor("featl", [R, D], F32, kind="ExternalInput")
    predl = nc.dram_tensor("predl", [R, C], F32, kind="ExternalInput")
    lab_all = nc.dram_tensor("lab_all", [B], F32, kind="ExternalInput")
    lab_loc = nc.dram_tensor("lab_loc", [R, 1], F32, kind="ExternalInput")
    tgt_loc = nc.dram_tensor("tgt_loc", [R, 1], F32, kind="ExternalInput")
    pfi = nc.dram_tensor("pfi", [KP, D], F32, kind="ExternalInput")
    pfp = nc.dram_tensor("pfp", [KP, D], F32, kind="ExternalInput")
    pidx = nc.dram_tensor("pidx", [128, nt_p * 8], mybir.dt.int16,
                          kind="ExternalInput")
    pvalid = nc.dram_tensor("pvalid", [128, nt_p], F32, kind="ExternalInput")
    acc_out = nc.dram_tensor("acc_out", [128, NCOL], F32, kind="ExternalOutput")

    def bcast_ap(handle, n):
        a = handle.ap() if isinstance(handle, bass.DRamTensorHandle) \
            else handle[:, :]
        return bass.AP(tensor=a.tensor, offset=a.offset, ap=[[0, 128], [1, n]])

    with tile.TileContext(nc) as tc:
        with (
            tc.tile_pool(name="persist", bufs=1) as persist,
            tc.tile_pool(name="stream", bufs=3) as stream,
            tc.tile_pool(name="dense", bufs=2) as dense,
            tc.tile_pool(name="small", bufs=2) as small,
            tc.tile_pool(name="gpsum", bufs=2, space="PSUM") as gpsum,
            tc.tile_pool(name="dscratch", bufs=1, space="DRAM") as dscratch,
        ):
            dma = nc.sync

            # DRAM scratch as pool tiles so Tile tracks store->read deps
            dp_dram = dscratch.tile([R, B], F32, tag="dp")
            rrow_dram = dscratch.tile([16, 128], F32, tag="rrow")
            srow_dram = dscratch.tile([16, 128], F32, tag="srow")

            # ---------- constants / setup ----------
            zeros = persist.tile([128, B], F32)
            nc.gpsimd.memset(zeros, 0.0)
            acc = persist.tile([128, NCOL], F32)
            nc.vector.memset(acc, 0.0)
            iota_c = persist.tile([128, C], F32)
            nc.gpsimd.iota(iota_c, pattern=[[1, C]], base=0,
                           channel_multiplier=0,
                           allow_small_or_imprecise_dtypes=True)
            # identity for PE transpose
            iota_sq = persist.tile([128, 128], F32)
            nc.gpsimd.iota(iota_sq, pattern=[[1, 128]], base=0,
                           channel_multiplier=0,
                           allow_small_or_imprecise_dtypes=True)
            pid = persist.tile([128, 1], F32)
            nc.gpsimd.iota(pid, pattern=[[0, 1]], base=0,
                           channel_multiplier=1,
                           allow_small_or_imprecise_dtypes=True)
            ident = persist.tile([128, 128], F32)
            nc.vector.tensor_scalar(out=ident, in0=iota_sq, scalar1=pid,
                                    scalar2=None, op0=ALU.is_equal)

            # ---------- big persistent loads ----------
            ft = []          # featT tiles [128, B] x KT
            dma_ring = [nc.sync, nc.scalar, nc.sync, nc.scalar]
            for k in range(KT):
                t = persist.tile([128, B], mybir.dt.float32r, tag=f"ft{k}")
                dma_ring[k % 4].dma_start(
                    out=t, in_=featT.ap()[k * 128:(k + 1) * 128, :])
                ft.append(t)
            ftl = []         # featT local tiles [128, R] x KT
            for k in range(KT):
                t = persist.tile([128, R], mybir.dt.float32r, tag=f"ftl{k}")
                dma.dma_start(out=t, in_=featTl.ap()[k * 128:(k + 1) * 128, :])
                ftl.append(t)

            lab_b = persist.tile([128, B], F32)
            nc.gpsimd.dma_start(out=lab_b, in_=bcast_ap(lab_all, B))

            # ---------- r_all via square+accum over natural feature tiles ----
            rcols = persist.tile([128, 16], F32)
            for t_i in range(16):
                fn_t = stream.tile([128, D], F32, tag="fn")
                nc.gpsimd.dma_start(
                    out=fn_t, in_=featN.ap()[t_i * 128:(t_i + 1) * 128, :])
                nc.scalar.activation(out=fn_t, in_=fn_t, func=AF.Square,
                                     accum_out=rcols[:, t_i:t_i + 1])

            rho_cols = persist.tile([128, 16], F32)
            nc.scalar.activation(out=rho_cols, in_=rcols, func=AF.Sqrt)
            s_cols = persist.tile([128, 16], F32)
            nc.vector.reciprocal(out=s_cols, in_=rho_cols)

            # transpose rcols/s_cols -> [16,128] rows -> DRAM -> broadcast
            for src, dst_dram in ((rcols, rrow_dram), (s_cols, srow_dram)):
                ps = gpsum.tile([16, 128], F32, tag="gpsum_big")
                nc.tensor.transpose(ps, src, ident)
                row_sb = small.tile([16, 128], F32, tag="rowsb")
                nc.scalar.copy(out=row_sb, in_=ps)
                dma.dma_start(out=dst_dram[:], in_=row_sb)

            r_b = persist.tile([128, B], F32)
            nc.gpsimd.dma_start(out=r_b, in_=bcast_ap(rrow_dram, B))
            s_b = persist.tile([128, B], F32)
            nc.gpsimd.dma_start(out=s_b, in_=bcast_ap(srow_dram, B))

            # ---------- local row stats ----------
            rloc = persist.tile([128, RT], F32)
            for m in range(RT):
                fl_t = stream.tile([128, D], F32, tag="fn")
                dma.dma_start(out=fl_t,
                              in_=featl.ap()[m * 128:(m + 1) * 128, :])
                nc.scalar.activation(out=fl_t, in_=fl_t, func=AF.Square,
                                     accum_out=rloc[:, m:m + 1])
            rho_loc = persist.tile([128, RT], F32)
            nc.scalar.activation(out=rho_loc, in_=rloc, func=AF.Sqrt)
            s_loc = persist.tile([128, RT], F32)
            nc.vector.reciprocal(out=s_loc, in_=rho_loc)

            lab_l = persist.tile([128, RT], F32)
            dma.dma_start(out=lab_l[:, 0:1], in_=lab_loc.ap()[0:128, :])
            dma.dma_start(out=lab_l[:, 1:2], in_=lab_loc.ap()[128:256, :])

            # ---------- dense phase: G matmul + sim/dist ----------
            for m in range(RT):
                gps = gpsum.tile([128, B], F32, tag="gpsum_big")
                for nchunk in range(NCHUNK):
                    for k in range(KT):
                        nc.tensor.matmul(
                            gps[:, nchunk * 512:(nchunk + 1) * 512],
                            ftl[k][:, m * 128:(m + 1) * 128],
                            ft[k][:, nchunk * 512:(nchunk + 1) * 512],
                            start=(k == 0), stop=(k == KT - 1),
                        )
                # lmb = (lab_b == lab_i) * BIGMASK
                lmb = dense.tile([128, B], F32, tag="dwork")
                nc.vector.tensor_scalar(out=lmb, in0=lab_b,
                                        scalar1=lab_l[:, m:m + 1],
                                        scalar2=BIGMASK,
                                        op0=ALU.is_equal, op1=ALU.mult)
                # rbl = lmb + r_i + r_j
                rbl = dense.tile([128, B], F32, tag="dwork")
                nc.vector.scalar_tensor_tensor(
                    out=rbl, in0=lmb, scalar=rloc[:, m:m + 1], in1=r_b,
                    op0=ALU.add, op1=ALU.add)
                # d2m = -2*G + rbl
                d2m = dense.tile([128, B], F32, tag="dwork")
                nc.vector.scalar_tensor_tensor(
                    out=d2m, in0=gps, scalar=-2.0, in1=rbl,
                    op0=ALU.mult, op1=ALU.add)
                # sim = (G * s_i) * s_j   -- also accumulate neg term
                sim = dense.tile([128, B], F32, tag="dwork")
                nc.vector.scalar_tensor_tensor(
                    out=sim, in0=gps, scalar=s_loc[:, m:m + 1], in1=s_b,
                    op0=ALU.mult, op1=ALU.mult)
                nc.vector.scalar_tensor_tensor(
                    out=sim, in0=sim, scalar=0.5, in1=zeros,
                    op0=ALU.subtract, op1=ALU.min,
                    accum_out=acc[:, COL_NEG + m:COL_NEG + m + 1])
                # D' = sqrt(relu(d2m))
                d2r = dense.tile([128, B], F32, tag="dwork")
                nc.scalar.activation(out=d2r, in_=d2m, func=AF.Relu)
                dpt = dense.tile([128, B], F32, tag="dwork")
                nc.scalar.activation(out=dpt, in_=d2r, func=AF.Sqrt)
                nc.scalar.dma_start(out=dp_dram[m * 128:(m + 1) * 128, :],
                                     in_=dpt)
                # self-pair triplet term: sum min(D' - margin, 0)
                nc.vector.scalar_tensor_tensor(
                    out=dpt, in0=dpt, scalar=float(T_MARGIN), in1=zeros,
                    op0=ALU.subtract, op1=ALU.min,
                    accum_out=acc[:, COL_SELF + m:COL_SELF + m + 1])

            # ---------- pair smalls: ri, rp, G over pair features ----------
            pk = persist.tile([128, 3 * nt_p], F32)   # [d2 | ri | rp] columns
            pk_g = persist.tile([128, nt_p], F32)
            for g in range(nt_p):
                fi_t = stream.tile([128, D], F32, tag="pfi")
                nc.sync.dma_start(out=fi_t,
                                   in_=pfi.ap()[g * 128:(g + 1) * 128, :])
                fp_t = stream.tile([128, D], F32, tag="pfp")
                nc.scalar.dma_start(out=fp_t,
                                    in_=pfp.ap()[g * 128:(g + 1) * 128, :])
                scr_c = stream.tile([128, D], F32, tag="pscr")
                nc.vector.scalar_tensor_tensor(
                    out=scr_c, in0=fi_t, scalar=1.0, in1=fp_t,
                    op0=ALU.mult, op1=ALU.mult,
                    accum_out=pk_g[:, g:g + 1])
                nc.vector.scalar_tensor_tensor(
                    out=fi_t, in0=fi_t, scalar=1.0, in1=fi_t,
                    op0=ALU.mult, op1=ALU.mult,
                    accum_out=pk[:, nt_p + g:nt_p + g + 1])
                nc.vector.scalar_tensor_tensor(
                    out=fp_t, in0=fp_t, scalar=1.0, in1=fp_t,
                    op0=ALU.mult, op1=ALU.mult,
                    accum_out=pk[:, 2 * nt_p + g:2 * nt_p + g + 1])

            # d2 = ri + rp - 2*G  (into pk[:, 0:nt_p])
            t_neg2g = small.tile([128, nt_p], F32, tag="pneg2g")
            nc.vector.scalar_tensor_tensor(
                out=t_neg2g, in0=pk_g, scalar=-2.0,
                in1=pk[:, nt_p:2 * nt_p], op0=ALU.mult, op1=ALU.add)
            nc.vector.tensor_add(pk[:, 0:nt_p], t_neg2g,
                                 pk[:, 2 * nt_p:3 * nt_p])
            # roots of [d2 | ri | rp] in one ACT pass
            proots = persist.tile([128, 3 * nt_p], F32)
            nc.scalar.activation(out=proots, in_=pk, func=AF.Sqrt)
            # x = d + margin
            px = persist.tile([128, nt_p], F32)
            nc.vector.tensor_scalar(out=px, in0=proots[:, 0:nt_p],
                                    scalar1=float(T_MARGIN), scalar2=None,
                                    op0=ALU.add)
            # 1/sqrt(ri), 1/sqrt(rp)
            pinv = small.tile([128, 2 * nt_p], F32, tag="pinv")
            nc.vector.reciprocal(out=pinv, in_=proots[:, nt_p:3 * nt_p])
            # simp = G * inv_i * inv_p
            simp = small.tile([128, nt_p], F32, tag="simp")
            nc.vector.tensor_mul(simp, pk_g, pinv[:, 0:nt_p])
            nc.vector.tensor_mul(simp, simp, pinv[:, nt_p:2 * nt_p])

            pval_t = persist.tile([128, nt_p], F32)
            dma.dma_start(out=pval_t, in_=pvalid.ap())

            # pos pair loss: -ln(exp(simp/T) + 1e-8), masked, accumulated
            pexp = small.tile([128, nt_p], F32, tag="pexp")
            nc.scalar.activation(out=pexp, in_=simp, func=AF.Exp,
                                 scale=float(1.0 / TEMPERATURE))
            pexp1 = small.tile([128, nt_p], F32, tag="pexp1")
            nc.vector.tensor_scalar(out=pexp1, in0=pexp, scalar1=1e-8,
                                    scalar2=None, op0=ALU.add)
            pln = small.tile([128, nt_p], F32, tag="pln")
            nc.scalar.activation(out=pln, in_=pexp1, func=AF.Ln)
            nc.vector.scalar_tensor_tensor(
                out=pln, in0=pln, scalar=-1.0, in1=pval_t,
                op0=ALU.mult, op1=ALU.mult,
                accum_out=acc[:, COL_POS:COL_POS + 1])
            # neg correction: min(simp, 0.5) * valid
            pmc = small.tile([128, nt_p], F32, tag="pmc")
            nc.vector.tensor_scalar(out=pmc, in0=simp, scalar1=0.5,
                                    scalar2=None, op0=ALU.min)
            nc.vector.scalar_tensor_tensor(
                out=pmc, in0=pmc, scalar=1.0, in1=pval_t,
                op0=ALU.mult, op1=ALU.mult,
                accum_out=acc[:, COL_NCO:COL_NCO + 1])

            # ---------- pair row gather + triplet accumulation ----------
            idx_sb = persist.tile([128, nt_p * 8], mybir.dt.int16)
            dma.dma_start(out=idx_sb, in_=pidx.ap())
            for g in range(nt_p):
                grow = stream.tile([128, 1, B], F32, tag="grow")
                nc.gpsimd.dma_gather(
                    out_ap=grow,
                    in_ap=dp_dram[:, :],
                    idxs_ap=idx_sb[:, g * 8:(g + 1) * 8],
                    num_idxs=128,
                    num_idxs_reg=128,
                    elem_size=B,
                )
                tmp_t = small.tile([128, 1], F32, tag="ptrip")
                nc.vector.scalar_tensor_tensor(
                    out=grow[:, 0, :], in0=grow[:, 0, :], scalar=px[:, g:g + 1],
                    in1=zeros, op0=ALU.subtract, op1=ALU.min,
                    accum_out=tmp_t)
                nc.vector.tensor_mul(
                    acc[:, COL_PAIR + g:COL_PAIR + g + 1], tmp_t,
                    pval_t[:, g:g + 1])

            # ---------- focal + label smoothing ----------
            for m in range(RT):
                pred_t = stream.tile([128, C], F32, tag="pred")
                nc.scalar.dma_start(out=pred_t,
                                    in_=predl.ap()[m * 128:(m + 1) * 128, :])
                tgt_t = small.tile([128, 1], F32, tag="tgt")
                dma.dma_start(out=tgt_t,
                              in_=tgt_loc.ap()[m * 128:(m + 1) * 128, :])
                # sum(exp(pred)) -> lse (no max-subtraction; pred ~ randn)
                escr = stream.tile([128, C], F32, tag="escr")
                se = small.tile([128, 1], F32, tag="se")
                nc.scalar.activation(out=escr, in_=pred_t, func=AF.Exp,
                                     accum_out=se)
                lse = small.tile([128, 1], F32, tag="lse")
                nc.scalar.activation(out=lse, in_=se, func=AF.Ln)
                # sum(pred)
                spred = small.tile([128, 1], F32, tag="spred")
                nc.vector.scalar_tensor_tensor(
                    out=escr, in0=pred_t, scalar=1.0, in1=zeros[:, 0:C],
                    op0=ALU.mult, op1=ALU.add, accum_out=spred)
                # pred[target]
                tmask = stream.tile([128, C], F32, tag="tmask")
                nc.vector.tensor_scalar(out=tmask, in0=iota_c, scalar1=tgt_t,
                                        scalar2=None, op0=ALU.is_equal)
                ptgt = small.tile([128, 1], F32, tag="ptgt")
                nc.vector.scalar_tensor_tensor(
                    out=escr, in0=pred_t, scalar=1.0, in1=tmask,
                    op0=ALU.mult, op1=ALU.mult, accum_out=ptgt)
                # ce = lse - ptgt ; pt = exp(-ce)
                ce = small.tile([128, 1], F32, tag="ce")
                nc.vector.tensor_sub(ce, lse, ptgt)
                pt = small.tile([128, 1], F32, tag="pt")
                nc.scalar.activation(out=pt, in_=ce, func=AF.Exp, scale=-1.0)
                onept = small.tile([128, 1], F32, tag="onept")
                nc.vector.tensor_scalar(out=onept, in0=pt, scalar1=-1.0,
                                        scalar2=1.0, op0=ALU.mult, op1=ALU.add)
                f2 = small.tile([128, 1], F32, tag="f2")
                nc.vector.tensor_mul(f2, onept, onept)
                nc.vector.tensor_mul(
                    acc[:, COL_FOC + m:COL_FOC + m + 1], f2, ce)
                # ls_i = lse - OFF*spred - (0.9-OFF)*ptgt
                t1 = small.tile([128, 1], F32, tag="lst1")
                nc.vector.tensor_scalar(out=t1, in0=spred,
                                        scalar1=float(-OFF), scalar2=None,
                                        op0=ALU.mult)
                t2 = small.tile([128, 1], F32, tag="lst2")
                nc.vector.scalar_tensor_tensor(
                    out=t2, in0=ptgt,
                    scalar=float(-(1.0 - SMOOTHING - OFF)), in1=t1,
                    op0=ALU.mult, op1=ALU.add)
                nc.vector.tensor_add(
                    acc[:, COL_LS + m:COL_LS + m + 1], lse, t2)

            # ---------- writeback ----------
            dma.dma_start(out=acc_out.ap(), in_=acc)

    nc.compile()
    meta = dict(nt_p=nt_p, NCOL=NCOL, COL_NEG=COL_NEG, COL_SELF=COL_SELF,
                COL_PAIR=COL_PAIR, COL_POS=COL_POS, COL_NCO=COL_NCO,
                COL_FOC=COL_FOC, COL_LS=COL_LS)
    _BUILD_CACHE[key] = (nc, meta)
    return nc, meta


def _host_prep(pred, target, features):
    """Build the 8 per-core input maps."""
    pred = np.asarray(pred, dtype=np.float32)
    target = np.asarray(target)
    features = np.asarray(features, dtype=np.float32)
    labels = target.astype(np.int64)

    featT = np.ascontiguousarray(features.T)               # [D, B]
    lab_f32 = labels.astype(np.float32)

    # same-label non-self pairs, grouped by anchor's core
    order = np.argsort(labels, kind="stable")
    sorted_lab = labels[order]
    # boundaries of equal-label runs
    starts = np.flatnonzero(np.r_[True, sorted_lab[1:] != sorted_lab[:-1]])
    ends = np.r_[starts[1:], len(sorted_lab)]
    pairs_i, pairs_p = [], []
    for s, e in zip(starts, ends):
        if e - s < 2:
            continue
        members = order[s:e]
        ii, pp = np.meshgrid(members, members, indexing="ij")
        m = ii != pp
        pairs_i.append(ii[m])
        pairs_p.append(pp[m])
    if pairs_i:
        pairs_i = np.concatenate(pairs_i)
        pairs_p = np.concatenate(pairs_p)
    else:
        pairs_i = np.zeros((0,), np.int64)
        pairs_p = np.zeros((0,), np.int64)
    k_real = len(pairs_i)

    core_of = pairs_i // R
    per_core = [(pairs_i[core_of == c], pairs_p[core_of == c])
                for c in range(N_CORES)]
    kmax = max((len(a) for a, _ in per_core), default=0)
    nt_p = max(1, math.ceil(kmax / 128))
    KP = nt_p * 128

    e1 = np.zeros((D,), np.float32)
    e1[0] = 1.0

    in_maps = []
    for c in range(N_CORES):
        pi, pp = per_core[c]
        k = len(pi)
        fi = np.empty((KP, D), np.float32)
        fp = np.empty((KP, D), np.float32)
        fi[:k] = features[pi]
        fp[:k] = features[pp]
        fi[k:] = e1
        fp[k:] = e1
        rowidx = np.zeros((KP,), np.int16)
        rowidx[:k] = (pi % R).astype(np.int16)
        valid = np.zeros((KP,), np.float32)
        valid[:k] = 1.0
        # gather idx layout: unwrapped[j] = idx_tile[j%16, j//16], per tile g
        # [p, g*8+s] = rowidx[g*128 + s*16 + p]; replicated into all 8
        # GPSIMD core windows (HW reads its own 16-partition group)
        idx16 = rowidx.reshape(nt_p, 8, 16).transpose(2, 0, 1).reshape(16, -1)
        pidx = np.ascontiguousarray(np.tile(idx16, (8, 1)))
        # valid layout: [128, nt_p] with [p, g] = valid[g*128+p]
        pval = np.ascontiguousarray(valid.reshape(nt_p, 128).T)

        rows = slice(c * R, (c + 1) * R)
        in_maps.append({
            "featT": featT,
            "featN": features,
            "featTl": np.ascontiguousarray(featT[:, rows]),
            "featl": np.ascontiguousarray(features[rows]),
            "predl": np.ascontiguousarray(pred[rows]),
            "lab_all": lab_f32,
            "lab_loc": np.ascontiguousarray(lab_f32[rows, None]),
            "tgt_loc": np.ascontiguousarray(lab_f32[rows, None]),
            "pfi": fi,
            "pfp": fp,
            "pidx": pidx,
            "pvalid": pval,
        })
    return in_maps, nt_p, k_real


def _combine(results, meta, k_real):
    """Host-side scalar all-reduce + final loss combination."""
    nt_p = meta["nt_p"]
    accs = np.stack([r["acc_out"] for r in results]).astype(np.float64)
    tot = accs.sum(axis=(0, 1))                 # [NCOL]

    neg_dense = -(tot[meta["COL_NEG"]] + tot[meta["COL_NEG"] + 1])
    self_trip = -(tot[meta["COL_SELF"]] + tot[meta["COL_SELF"] + 1])
    pair_trip = -tot[meta["COL_PAIR"]:meta["COL_PAIR"] + nt_p].sum()
    pair_pos = tot[meta["COL_POS"]]
    negcorr_min = tot[meta["COL_NCO"]]
    focal_sum = tot[meta["COL_FOC"]] + tot[meta["COL_FOC"] + 1]
    ls_sum = tot[meta["COL_LS"]] + tot[meta["COL_LS"] + 1]

    k_tot = k_real + B
    pos_self = B * (-np.log(np.exp(1.0 / TEMPERATURE) + 1e-8))
    pos_zero = (B * B - k_tot) * (-np.log1p(1e-8))
    pos_sum = pair_pos + pos_self + pos_zero
    neg_sum = neg_dense + negcorr_min + 0.5 * B

    lc = (pos_sum + neg_sum) / (B * B)
    lt = (self_trip + pair_trip) / (B + 1e-8)
    lf = ALPHA * focal_sum / B
    ls = ls_sum / B
    total = (W_CONTRASTIVE * lc + W_TRIPLET * lt
             + W_FOCAL * lf + W_LABEL_SMOOTH * ls)
    return np.array([lc, lt, lf, ls, total], dtype=np.float32)


def kernel(pred, target, features):
    in_maps, nt_p, k_real = _host_prep(pred, target, features)
    # fix the tgt entries (they must be the class targets, same as labels here)
    nc, meta = _build(nt_p)
    res = run_bass_kernel_spmd(nc, in_maps, core_ids=list(range(N_CORES)))
    return _combine(res.results, meta, k_real)


if __name__ == "__main__":
    import reference

    inputs = reference.setup_inputs()
    expected = np.asarray(reference.reference(**inputs))
    actual = kernel(**{k: np.asarray(v) for k, v in inputs.items()})
    err = np.abs(actual - expected) / np.maximum(np.abs(expected), 1e-12)
    print("expected:", expected)
    print("actual:  ", actual)
    print("rel err: ", err)


# revision 12
# speedup vs baseline: 1.1299x; 1.0260x over previous
"""Trainium2 Bass kernel for nn_EnhancedLossModule (contrastive + triplet +
focal + label-smoothing loss over B=2048, C=1000, D=512).

Strategy (8 NeuronCores, SPMD):
  - Shard rows of the [B,B] gram/distance matrices: 256 rows per core.
  - One fp32 matmul per core G = f_local @ f_all^T ([256,2048]) feeds both the
    contrastive similarity (sim = G*s_i*s_j) and squared distances
    (d^2 = r_i + r_j - 2G).
  - Triplet loss: sum over same-label (anchor, positive) pairs of
    sum_n relu(d_ap + margin - d_an) * [label_n != label_a].  The mask is
    folded into the dense distance matrix (same-label entries pushed to a
    huge value so the relu kills them).  Self-pairs (a==p) read the dense
    rows directly; real pairs gather their anchor rows via dma_gather.
  - Focal + label smoothing: data-parallel over the pred shard.
  - Each core DMAs out a [128, NCOL] tile of per-partition partial sums;
    the host sums them (the scalar "all-reduce") and combines the losses.
"""

import math

import ml_dtypes
import numpy as np

import concourse.bacc as bacc
import concourse.bass as bass
import concourse.tile as tile
from concourse import mybir
from concourse.bass_utils import run_bass_kernel_spmd

# ---- problem constants (hardcoded per the task spec) ----
B, C, D = 2048, 1000, 512
N_CORES = 8
R = B // N_CORES          # rows per core = 256
RT = R // 128             # row tiles per core = 2
KT = D // 128             # contraction tiles = 4
NCHUNK = 4                # 2048 / 512 psum chunks

TEMPERATURE = 0.07
C_MARGIN = 0.5
T_MARGIN = 1.0
GAMMA = 2.0
ALPHA = 0.25
SMOOTHING = 0.1
W_CONTRASTIVE = 0.1
W_TRIPLET = 0.1
W_FOCAL = 0.4
W_LABEL_SMOOTH = 0.4

BIGMASK = 1.0e9           # added to d^2 of same-label entries
OFF = SMOOTHING / (C - 1)

F32 = mybir.dt.float32
ALU = mybir.AluOpType
AF = mybir.ActivationFunctionType

_BUILD_CACHE: dict = {}


def _build(nt_p: int):
    """Build + compile the SPMD bass program for nt_p pair tiles per core."""
    key = nt_p
    if key in _BUILD_CACHE:
        return _BUILD_CACHE[key]

    KP = nt_p * 128

    # accumulator column map
    COL_NEG = 0                    # 2 cols: sum min(sim-0.5, 0), per row tile
    COL_SELF = 2                   # 2 cols: sum min(D'-margin, 0), per row tile
    COL_PAIR = 4                   # nt_p cols: valid*sum min(D'row-x, 0)
    COL_POS = COL_PAIR + nt_p      # 1 col: sum valid * -ln(exp(sim/T)+eps)
    COL_NCO = COL_POS + 1          # 1 col: sum valid * min(simp, 0.5)
    COL_FOC = COL_NCO + 1          # 2 cols: sum (1-pt)^2*ce, per row tile
    COL_LS = COL_FOC + 2           # 2 cols: label-smoothing per row tile
    NCOL = COL_LS + 2

    nc = bacc.Bacc(
        "TRN2", target_bir_lowering=False, debug=False, num_devices=N_CORES
    )

    # ---- DRAM I/O ----
    featT = nc.dram_tensor("featT", [D, B], mybir.dt.bfloat16,
                           kind="ExternalInput")
    featN = nc.dram_tensor("featN", [B, D], F32, kind="ExternalInput")
    featTl = nc.dram_tensor("featTl", [D, R], mybir.dt.bfloat16,
                            kind="ExternalInput")
    featl = nc.dram_tensor("featl", [R, D], F32, kind="ExternalInput")
    predl = nc.dram_tensor("predl", [R, C], F32, kind="ExternalInput")
    lab_all = nc.dram_tensor("lab_all", [B], F32, kind="ExternalInput")
    lab_loc = nc.dram_tensor("lab_loc", [R, 1], F32, kind="ExternalInput")
    tgt_loc = nc.dram_tensor("tgt_loc", [R, 1], F32, kind="ExternalInput")
    pfi = nc.dram_tensor("pfi", [KP, D], F32, kind="ExternalInput")
    pfp = nc.dram_tensor("pfp", [KP, D], F32, kind="ExternalInput")
    pidx = nc.dram_tensor("pidx", [128, nt_p * 8], mybir.dt.int16,
                          kind="ExternalInput")
    pvalid = nc.dram_tensor("pvalid", [128, nt_p], F32, kind="ExternalInput")
    acc_out = nc.dram_tensor("acc_out", [128, NCOL], F32, kind="ExternalOutput")

    def bcast_ap(handle, n):
        a = handle.ap() if isinstance(handle, bass.DRamTensorHandle) \
            else handle[:, :]
        return bass.AP(tensor=a.tensor, offset=a.offset, ap=[[0, 128], [1, n]])

    with tile.TileContext(nc) as tc:
        with (
            tc.tile_pool(name="persist", bufs=1) as persist,
            tc.tile_pool(name="stream", bufs=3) as stream,
            tc.tile_pool(name="dense", bufs=2) as dense,
            tc.tile_pool(name="small", bufs=2) as small,
            tc.tile_pool(name="gpsum", bufs=2, space="PSUM") as gpsum,
            tc.tile_pool(name="dscratch", bufs=1, space="DRAM") as dscratch,
        ):
            dma = nc.sync

            # DRAM scratch as pool tiles so Tile tracks store->read deps
            dp_dram = dscratch.tile([R, B], F32, tag="dp")
            rrow_dram = dscratch.tile([16, 128], F32, tag="rrow")
            srow_dram = dscratch.tile([16, 128], F32, tag="srow")

            # ---------- constants / setup ----------
            zeros = persist.tile([128, B], F32)
            nc.gpsimd.memset(zeros, 0.0)
            acc = persist.tile([128, NCOL], F32)
            nc.vector.memset(acc, 0.0)
            iota_c = persist.tile([128, C], F32)
            nc.gpsimd.iota(iota_c, pattern=[[1, C]], base=0,
                           channel_multiplier=0,
                           allow_small_or_imprecise_dtypes=True)
            # identity for PE transpose
            iota_sq = persist.tile([128, 128], F32)
            nc.gpsimd.iota(iota_sq, pattern=[[1, 128]], base=0,
                           channel_multiplier=0,
                           allow_small_or_imprecise_dtypes=True)
            pid = persist.tile([128, 1], F32)
            nc.gpsimd.iota(pid, pattern=[[0, 1]], base=0,
                           channel_multiplier=1,
                           allow_small_or_imprecise_dtypes=True)
            ident = persist.tile([128, 128], F32)
            nc.vector.tensor_scalar(out=ident, in0=iota_sq, scalar1=pid,
                                    scalar2=None, op0=ALU.is_equal)

            # ---------- big persistent loads ----------
            ft = []          # featT tiles [128, B] x KT
            dma_ring = [nc.sync, nc.scalar, nc.sync, nc.scalar]
            for k in range(KT):
                t = persist.tile([128, B], mybir.dt.bfloat16, tag=f"ft{k}")
                dma_ring[k % 4].dma_start(
                    out=t, in_=featT.ap()[k * 128:(k + 1) * 128, :])
                ft.append(t)
            ftl = []         # featT local tiles [128, R] x KT
            for k in range(KT):
                t = persist.tile([128, R], mybir.dt.bfloat16, tag=f"ftl{k}")
                dma.dma_start(out=t, in_=featTl.ap()[k * 128:(k + 1) * 128, :])
                ftl.append(t)

            lab_b = persist.tile([128, B], F32)
            nc.gpsimd.dma_start(out=lab_b, in_=bcast_ap(lab_all, B))

            # ---------- r_all via square+accum over natural feature tiles ----
            rcols = persist.tile([128, 16], F32)
            for t_i in range(16):
                fn_t = stream.tile([128, D], F32, tag="fn")
                dma.dma_start(
                    out=fn_t, in_=featN.ap()[t_i * 128:(t_i + 1) * 128, :])
                nc.scalar.activation(out=fn_t, in_=fn_t, func=AF.Square,
                                     accum_out=rcols[:, t_i:t_i + 1])

            rho_cols = persist.tile([128, 16], F32)
            nc.scalar.activation(out=rho_cols, in_=rcols, func=AF.Sqrt)
            s_cols = persist.tile([128, 16], F32)
            nc.vector.reciprocal(out=s_cols, in_=rho_cols)

            # transpose rcols/s_cols -> [16,128] rows -> DRAM -> broadcast
            for src, dst_dram in ((rcols, rrow_dram), (s_cols, srow_dram)):
                ps = gpsum.tile([16, 128], F32, tag="gpsum_big")
                nc.tensor.transpose(ps, src, ident)
                row_sb = small.tile([16, 128], F32, tag="rowsb")
                nc.scalar.copy(out=row_sb, in_=ps)
                dma.dma_start(out=dst_dram[:], in_=row_sb)

            r_b = persist.tile([128, B], F32)
            nc.gpsimd.dma_start(out=r_b, in_=bcast_ap(rrow_dram, B))
            s_b = persist.tile([128, B], F32)
            nc.gpsimd.dma_start(out=s_b, in_=bcast_ap(srow_dram, B))

            # ---------- local row stats ----------
            rloc = persist.tile([128, RT], F32)
            for m in range(RT):
                fl_t = stream.tile([128, D], F32, tag="fn")
                dma.dma_start(out=fl_t,
                              in_=featl.ap()[m * 128:(m + 1) * 128, :])
                nc.scalar.activation(out=fl_t, in_=fl_t, func=AF.Square,
                                     accum_out=rloc[:, m:m + 1])
            rho_loc = persist.tile([128, RT], F32)
            nc.scalar.activation(out=rho_loc, in_=rloc, func=AF.Sqrt)
            s_loc = persist.tile([128, RT], F32)
            nc.vector.reciprocal(out=s_loc, in_=rho_loc)

            lab_l = persist.tile([128, RT], F32)
            dma.dma_start(out=lab_l[:, 0:1], in_=lab_loc.ap()[0:128, :])
            dma.dma_start(out=lab_l[:, 1:2], in_=lab_loc.ap()[128:256, :])

            # ---------- dense phase: G matmul + sim/dist ----------
            for m in range(RT):
                gps = gpsum.tile([128, B], F32, tag="gpsum_big")
                for nchunk in range(NCHUNK):
                    for k in range(KT):
                        nc.tensor.matmul(
                            gps[:, nchunk * 512:(nchunk + 1) * 512],
                            ftl[k][:, m * 128:(m + 1) * 128],
                            ft[k][:, nchunk * 512:(nchunk + 1) * 512],
                            start=(k == 0), stop=(k == KT - 1),
                        )
                # lmb = (lab_b == lab_i) * BIGMASK
                lmb = dense.tile([128, B], F32, tag="dwork")
                nc.vector.tensor_scalar(out=lmb, in0=lab_b,
                                        scalar1=lab_l[:, m:m + 1],
                                        scalar2=BIGMASK,
                                        op0=ALU.is_equal, op1=ALU.mult)
                # rbl = lmb + r_i + r_j
                rbl = dense.tile([128, B], F32, tag="dwork")
                nc.vector.scalar_tensor_tensor(
                    out=rbl, in0=lmb, scalar=rloc[:, m:m + 1], in1=r_b,
                    op0=ALU.add, op1=ALU.add)
                # d2m = -2*G + rbl
                d2m = dense.tile([128, B], F32, tag="dwork")
                nc.vector.scalar_tensor_tensor(
                    out=d2m, in0=gps, scalar=-2.0, in1=rbl,
                    op0=ALU.mult, op1=ALU.add)
                # sim = (G * s_i) * s_j   -- also accumulate neg term
                sim = dense.tile([128, B], F32, tag="dwork")
                nc.vector.scalar_tensor_tensor(
                    out=sim, in0=gps, scalar=s_loc[:, m:m + 1], in1=s_b,
                    op0=ALU.mult, op1=ALU.mult)
                nc.vector.scalar_tensor_tensor(
                    out=sim, in0=sim, scalar=0.5, in1=zeros,
                    op0=ALU.subtract, op1=ALU.min,
                    accum_out=acc[:, COL_NEG + m:COL_NEG + m + 1])
                # D' = sqrt(relu(d2m))
                d2r = dense.tile([128, B], F32, tag="dwork")
                nc.scalar.activation(out=d2r, in_=d2m, func=AF.Relu)
                dpt = dense.tile([128, B], F32, tag="dwork")
                nc.scalar.activation(out=dpt, in_=d2r, func=AF.Sqrt)
                nc.scalar.dma_start(out=dp_dram[m * 128:(m + 1) * 128, :],
                                     in_=dpt)
                # self-pair triplet term: sum min(D' - margin, 0)
                nc.vector.scalar_tensor_tensor(
                    out=dpt, in0=dpt, scalar=float(T_MARGIN), in1=zeros,
                    op0=ALU.subtract, op1=ALU.min,
                    accum_out=acc[:, COL_SELF + m:COL_SELF + m + 1])

            # ---------- pair smalls: ri, rp, G over pair features ----------
            pk = persist.tile([128, 3 * nt_p], F32)   # [d2 | ri | rp] columns
            pk_g = persist.tile([128, nt_p], F32)
            for g in range(nt_p):
                fi_t = stream.tile([128, D], F32, tag="pfi")
                nc.sync.dma_start(out=fi_t,
                                   in_=pfi.ap()[g * 128:(g + 1) * 128, :])
                fp_t = stream.tile([128, D], F32, tag="pfp")
                nc.scalar.dma_start(out=fp_t,
                                    in_=pfp.ap()[g * 128:(g + 1) * 128, :])
                scr_c = stream.tile([128, D], F32, tag="pscr")
                nc.vector.scalar_tensor_tensor(
                    out=scr_c, in0=fi_t, scalar=1.0, in1=fp_t,
                    op0=ALU.mult, op1=ALU.mult,
                    accum_out=pk_g[:, g:g + 1])
                nc.vector.scalar_tensor_tensor(
                    out=fi_t, in0=fi_t, scalar=1.0, in1=fi_t,
                    op0=ALU.mult, op1=ALU.mult,
                    accum_out=pk[:, nt_p + g:nt_p + g + 1])
                nc.vector.scalar_tensor_tensor(
                    out=fp_t, in0=fp_t, scalar=1.0, in1=fp_t,
                    op0=ALU.mult, op1=ALU.mult,
                    accum_out=pk[:, 2 * nt_p + g:2 * nt_p + g + 1])

            # d2 = ri + rp - 2*G  (into pk[:, 0:nt_p])
            t_neg2g = small.tile([128, nt_p], F32, tag="pneg2g")
            nc.vector.scalar_tensor_tensor(
                out=t_neg2g, in0=pk_g, scalar=-2.0,
                in1=pk[:, nt_p:2 * nt_p], op0=ALU.mult, op1=ALU.add)
            nc.vector.tensor_add(pk[:, 0:nt_p], t_neg2g,
                                 pk[:, 2 * nt_p:3 * nt_p])
            # roots of [d2 | ri | rp] in one ACT pass
            proots = persist.tile([128, 3 * nt_p], F32)
            nc.scalar.activation(out=proots, in_=pk, func=AF.Sqrt)
            # x = d + margin
            px = persist.tile([128, nt_p], F32)
            nc.vector.tensor_scalar(out=px, in0=proots[:, 0:nt_p],
                                    scalar1=float(T_MARGIN), scalar2=None,
                                    op0=ALU.add)
            # 1/sqrt(ri), 1/sqrt(rp)
            pinv = small.tile([128, 2 * nt_p], F32, tag="pinv")
            nc.vector.reciprocal(out=pinv, in_=proots[:, nt_p:3 * nt_p])
            # simp = G * inv_i * inv_p
            simp = small.tile([128, nt_p], F32, tag="simp")
            nc.vector.tensor_mul(simp, pk_g, pinv[:, 0:nt_p])
            nc.vector.tensor_mul(simp, simp, pinv[:, nt_p:2 * nt_p])

            pval_t = persist.tile([128, nt_p], F32)
            dma.dma_start(out=pval_t, in_=pvalid.ap())

            # pos pair loss: -ln(exp(simp/T) + 1e-8), masked, accumulated
            pexp = small.tile([128, nt_p], F32, tag="pexp")
            nc.scalar.activation(out=pexp, in_=simp, func=AF.Exp,
                                 scale=float(1.0 / TEMPERATURE))
            pexp1 = small.tile([128, nt_p], F32, tag="pexp1")
            nc.vector.tensor_scalar(out=pexp1, in0=pexp, scalar1=1e-8,
                                    scalar2=None, op0=ALU.add)
            pln = small.tile([128, nt_p], F32, tag="pln")
            nc.scalar.activation(out=pln, in_=pexp1, func=AF.Ln)
            nc.vector.scalar_tensor_tensor(
                out=pln, in0=pln, scalar=-1.0, in1=pval_t,
                op0=ALU.mult, op1=ALU.mult,
                accum_out=acc[:, COL_POS:COL_POS + 1])
            # neg correction: min(simp, 0.5) * valid
            pmc = small.tile([128, nt_p], F32, tag="pmc")
            nc.vector.tensor_scalar(out=pmc, in0=simp, scalar1=0.5,
                                    scalar2=None, op0=ALU.min)
            nc.vector.scalar_tensor_tensor(
                out=pmc, in0=pmc, scalar=1.0, in1=pval_t,
                op0=ALU.mult, op1=ALU.mult,
                accum_out=acc[:, COL_NCO:COL_NCO + 1])

            # ---------- pair row gather + triplet accumulation ----------
            idx_sb = persist.tile([128, nt_p * 8], mybir.dt.int16)
            dma.dma_start(out=idx_sb, in_=pidx.ap())
            for g in range(nt_p):
                grow = stream.tile([128, 1, B], F32, tag="grow")
                nc.gpsimd.dma_gather(
                    out_ap=grow,
                    in_ap=dp_dram[:, :],
                    idxs_ap=idx_sb[:, g * 8:(g + 1) * 8],
                    num_idxs=128,
                    num_idxs_reg=128,
                    elem_size=B,
                )
                tmp_t = small.tile([128, 1], F32, tag="ptrip")
                nc.vector.scalar_tensor_tensor(
                    out=grow[:, 0, :], in0=grow[:, 0, :], scalar=px[:, g:g + 1],
                    in1=zeros, op0=ALU.subtract, op1=ALU.min,
                    accum_out=tmp_t)
                nc.vector.tensor_mul(
                    acc[:, COL_PAIR + g:COL_PAIR + g + 1], tmp_t,
                    pval_t[:, g:g + 1])

            # ---------- focal + label smoothing ----------
            for m in range(RT):
                pred_t = stream.tile([128, C], F32, tag="pred")
                nc.scalar.dma_start(out=pred_t,
                                    in_=predl.ap()[m * 128:(m + 1) * 128, :])
                tgt_t = small.tile([128, 1], F32, tag="tgt")
                dma.dma_start(out=tgt_t,
                              in_=tgt_loc.ap()[m * 128:(m + 1) * 128, :])
                # sum(exp(pred)) -> lse (no max-subtraction; pred ~ randn)
                escr = stream.tile([128, C], F32, tag="escr")
                se = small.tile([128, 1], F32, tag="se")
                nc.scalar.activation(out=escr, in_=pred_t, func=AF.Exp,
                                     accum_out=se)
                lse = small.tile([128, 1], F32, tag="lse")
                nc.scalar.activation(out=lse, in_=se, func=AF.Ln)
                # sum(pred)
                spred = small.tile([128, 1], F32, tag="spred")
                nc.vector.scalar_tensor_tensor(
                    out=escr, in0=pred_t, scalar=1.0, in1=zeros[:, 0:C],
                    op0=ALU.mult, op1=ALU.add, accum_out=spred)
                # pred[target]
                tmask = stream.tile([128, C], F32, tag="tmask")
                nc.vector.tensor_scalar(out=tmask, in0=iota_c, scalar1=tgt_t,
                                        scalar2=None, op0=ALU.is_equal)
                ptgt = small.tile([128, 1], F32, tag="ptgt")
                nc.vector.scalar_tensor_tensor(
                    out=escr, in0=pred_t, scalar=1.0, in1=tmask,
                    op0=ALU.mult, op1=ALU.mult, accum_out=ptgt)
                # ce = lse - ptgt ; pt = exp(-ce)
                ce = small.tile([128, 1], F32, tag="ce")
                nc.vector.tensor_sub(ce, lse, ptgt)
                pt = small.tile([128, 1], F32, tag="pt")
                nc.scalar.activation(out=pt, in_=ce, func=AF.Exp, scale=-1.0)
                onept = small.tile([128, 1], F32, tag="onept")
                nc.vector.tensor_scalar(out=onept, in0=pt, scalar1=-1.0,
                                        scalar2=1.0, op0=ALU.mult, op1=ALU.add)
                f2 = small.tile([128, 1], F32, tag="f2")
                nc.vector.tensor_mul(f2, onept, onept)
                nc.vector.tensor_mul(
                    acc[:, COL_FOC + m:COL_FOC + m + 1], f2, ce)
                # ls_i = lse - OFF*spred - (0.9-OFF)*ptgt
                t1 = small.tile([128, 1], F32, tag="lst1")
                nc.vector.tensor_scalar(out=t1, in0=spred,
                                        scalar1=float(-OFF), scalar2=None,
                                        op0=ALU.mult)
                t2 = small.tile([128, 1], F32, tag="lst2")
                nc.vector.scalar_tensor_tensor(
                    out=t2, in0=ptgt,
                    scalar=float(-(1.0 - SMOOTHING - OFF)), in1=t1,
                    op0=ALU.mult, op1=ALU.add)
                nc.vector.tensor_add(
                    acc[:, COL_LS + m:COL_LS + m + 1], lse, t2)

            # ---------- writeback ----------
            dma.dma_start(out=acc_out.ap(), in_=acc)

    nc.compile()
    meta = dict(nt_p=nt_p, NCOL=NCOL, COL_NEG=COL_NEG, COL_SELF=COL_SELF,
                COL_PAIR=COL_PAIR, COL_POS=COL_POS, COL_NCO=COL_NCO,
                COL_FOC=COL_FOC, COL_LS=COL_LS)
    _BUILD_CACHE[key] = (nc, meta)
    return nc, meta


def _host_prep(pred, target, features):
    """Build the 8 per-core input maps."""
    pred = np.asarray(pred, dtype=np.float32)
    target = np.asarray(target)
    features = np.asarray(features, dtype=np.float32)
    labels = target.astype(np.int64)

    featT = np.ascontiguousarray(features.T)               # [D, B]
    featT_bf = featT.astype(ml_dtypes.bfloat16)
    lab_f32 = labels.astype(np.float32)

    # same-label non-self pairs, grouped by anchor's core
    order = np.argsort(labels, kind="stable")
    sorted_lab = labels[order]
    # boundaries of equal-label runs
    starts = np.flatnonzero(np.r_[True, sorted_lab[1:] != sorted_lab[:-1]])
    ends = np.r_[starts[1:], len(sorted_lab)]
    pairs_i, pairs_p = [], []
    for s, e in zip(starts, ends):
        if e - s < 2:
            continue
        members = order[s:e]
        ii, pp = np.meshgrid(members, members, indexing="ij")
        m = ii != pp
        pairs_i.append(ii[m])
        pairs_p.append(pp[m])
    if pairs_i:
        pairs_i = np.concatenate(pairs_i)
        pairs_p = np.concatenate(pairs_p)
    else:
        pairs_i = np.zeros((0,), np.int64)
        pairs_p = np.zeros((0,), np.int64)
    k_real = len(pairs_i)

    core_of = pairs_i // R
    per_core = [(pairs_i[core_of == c], pairs_p[core_of == c])
                for c in range(N_CORES)]
    kmax = max((len(a) for a, _ in per_core), default=0)
    nt_p = max(1, math.ceil(kmax / 128))
    KP = nt_p * 128

    e1 = np.zeros((D,), np.float32)
    e1[0] = 1.0

    in_maps = []
    for c in range(N_CORES):
        pi, pp = per_core[c]
        k = len(pi)
        fi = np.empty((KP, D), np.float32)
        fp = np.empty((KP, D), np.float32)
        fi[:k] = features[pi]
        fp[:k] = features[pp]
        fi[k:] = e1
        fp[k:] = e1
        rowidx = np.zeros((KP,), np.int16)
        rowidx[:k] = (pi % R).astype(np.int16)
        valid = np.zeros((KP,), np.float32)
        valid[:k] = 1.0
        # gather idx layout: unwrapped[j] = idx_tile[j%16, j//16], per tile g
        # [p, g*8+s] = rowidx[g*128 + s*16 + p]; replicated into all 8
        # GPSIMD core windows (HW reads its own 16-partition group)
        idx16 = rowidx.reshape(nt_p, 8, 16).transpose(2, 0, 1).reshape(16, -1)
        pidx = np.ascontiguousarray(np.tile(idx16, (8, 1)))
        # valid layout: [128, nt_p] with [p, g] = valid[g*128+p]
        pval = np.ascontiguousarray(valid.reshape(nt_p, 128).T)

        rows = slice(c * R, (c + 1) * R)
        in_maps.append({
            "featT": featT_bf,
            "featN": features,
            "featTl": np.ascontiguousarray(featT_bf[:, rows]),
            "featl": np.ascontiguousarray(features[rows]),
            "predl": np.ascontiguousarray(pred[rows]),
            "lab_all": lab_f32,
            "lab_loc": np.ascontiguousarray(lab_f32[rows, None]),
            "tgt_loc": np.ascontiguousarray(lab_f32[rows, None]),
            "pfi": fi,
            "pfp": fp,
            "pidx": pidx,
            "pvalid": pval,
        })
    return in_maps, nt_p, k_real


def _combine(results, meta, k_real):
    """Host-side scalar all-reduce + final loss combination."""
    nt_p = meta["nt_p"]
    accs = np.stack([r["acc_out"] for r in results]).astype(np.float64)
    tot = accs.sum(axis=(0, 1))                 # [NCOL]

    neg_dense = -(tot[meta["COL_NEG"]] + tot[meta["COL_NEG"] + 1])
    self_trip = -(tot[meta["COL_SELF"]] + tot[meta["COL_SELF"] + 1])
    pair_trip = -tot[meta["COL_PAIR"]:meta["COL_PAIR"] + nt_p].sum()
    pair_pos = tot[meta["COL_POS"]]
    negcorr_min = tot[meta["COL_NCO"]]
    focal_sum = tot[meta["COL_FOC"]] + tot[meta["COL_FOC"] + 1]
    ls_sum = tot[meta["COL_LS"]] + tot[meta["COL_LS"] + 1]

    k_tot = k_real + B
    pos_self = B * (-np.log(np.exp(1.0 / TEMPERATURE) + 1e-8))
    pos_zero = (B * B - k_tot) * (-np.log1p(1e-8))
    pos_sum = pair_pos + pos_self + pos_zero
    neg_sum = neg_dense + negcorr_min + 0.5 * B

    lc = (pos_sum + neg_sum) / (B * B)
    lt = (self_trip + pair_trip) / (B + 1e-8)
    lf = ALPHA * focal_sum / B
    ls = ls_sum / B
    total = (W_CONTRASTIVE * lc + W_TRIPLET * lt
             + W_FOCAL * lf + W_LABEL_SMOOTH * ls)
    return np.array([lc, lt, lf, ls, total], dtype=np.float32)


def kernel(pred, target, features):
    in_maps, nt_p, k_real = _host_prep(pred, target, features)
    # fix the tgt entries (they must be the class targets, same as labels here)
    nc, meta = _build(nt_p)
    res = run_bass_kernel_spmd(nc, in_maps, core_ids=list(range(N_CORES)))
    return _combine(res.results, meta, k_real)


if __name__ == "__main__":
    import reference

    inputs = reference.setup_inputs()
    expected = np.asarray(reference.reference(**inputs))
    actual = kernel(**{k: np.asarray(v) for k, v in inputs.items()})
    err = np.abs(actual - expected) / np.maximum(np.abs(expected), 1e-12)
    print("expected:", expected)
    print("actual:  ", actual)
    print("rel err: ", err)


# revision 13
# speedup vs baseline: 1.1624x; 1.0287x over previous
"""Trainium2 Bass kernel for nn_EnhancedLossModule (contrastive + triplet +
focal + label-smoothing loss over B=2048, C=1000, D=512).

Strategy (8 NeuronCores, SPMD):
  - Shard rows of the [B,B] gram/distance matrices: 256 rows per core.
  - One fp32 matmul per core G = f_local @ f_all^T ([256,2048]) feeds both the
    contrastive similarity (sim = G*s_i*s_j) and squared distances
    (d^2 = r_i + r_j - 2G).
  - Triplet loss: sum over same-label (anchor, positive) pairs of
    sum_n relu(d_ap + margin - d_an) * [label_n != label_a].  The mask is
    folded into the dense distance matrix (same-label entries pushed to a
    huge value so the relu kills them).  Self-pairs (a==p) read the dense
    rows directly; real pairs gather their anchor rows via dma_gather.
  - Focal + label smoothing: data-parallel over the pred shard.
  - Each core DMAs out a [128, NCOL] tile of per-partition partial sums;
    the host sums them (the scalar "all-reduce") and combines the losses.
"""

import math

import ml_dtypes
import numpy as np

import concourse.bacc as bacc
import concourse.bass as bass
import concourse.tile as tile
from concourse import mybir
from concourse.bass_utils import run_bass_kernel_spmd

# ---- problem constants (hardcoded per the task spec) ----
B, C, D = 2048, 1000, 512
N_CORES = 8
R = B // N_CORES          # rows per core = 256
RT = R // 128             # row tiles per core = 2
KT = D // 128             # contraction tiles = 4
NCHUNK = 4                # 2048 / 512 psum chunks

TEMPERATURE = 0.07
C_MARGIN = 0.5
T_MARGIN = 1.0
GAMMA = 2.0
ALPHA = 0.25
SMOOTHING = 0.1
W_CONTRASTIVE = 0.1
W_TRIPLET = 0.1
W_FOCAL = 0.4
W_LABEL_SMOOTH = 0.4

BIGMASK = 1.0e9           # added to d^2 of same-label entries
OFF = SMOOTHING / (C - 1)

F32 = mybir.dt.float32
ALU = mybir.AluOpType
AF = mybir.ActivationFunctionType

_BUILD_CACHE: dict = {}


def _build(nt_p: int):
    """Build + compile the SPMD bass program for nt_p pair tiles per core."""
    key = nt_p
    if key in _BUILD_CACHE:
        return _BUILD_CACHE[key]

    KP = nt_p * 128

    # accumulator column map
    COL_NEG = 0                    # 2 cols: sum min(sim-0.5, 0), per row tile
    COL_SELF = 2                   # 2 cols: sum min(D'-margin, 0), per row tile
    COL_PAIR = 4                   # nt_p cols: valid*sum min(D'row-x, 0)
    COL_POS = COL_PAIR + nt_p      # 1 col: sum valid * -ln(exp(sim/T)+eps)
    COL_NCO = COL_POS + 1          # 1 col: sum valid * min(simp, 0.5)
    COL_FOC = COL_NCO + 1          # 2 cols: sum (1-pt)^2*ce, per row tile
    COL_LS = COL_FOC + 2           # 2 cols: label-smoothing per row tile
    NCOL = COL_LS + 2

    nc = bacc.Bacc(
        "TRN2", target_bir_lowering=False, debug=False, num_devices=N_CORES
    )

    # ---- DRAM I/O ----
    featT = nc.dram_tensor("featT", [D, B], mybir.dt.bfloat16,
                           kind="ExternalInput")
    featN = nc.dram_tensor("featN", [B, D], F32, kind="ExternalInput")
    featTl = nc.dram_tensor("featTl", [D, R], mybir.dt.bfloat16,
                            kind="ExternalInput")
    featl = nc.dram_tensor("featl", [R, D], F32, kind="ExternalInput")
    predl = nc.dram_tensor("predl", [R, C], F32, kind="ExternalInput")
    lab_all = nc.dram_tensor("lab_all", [B], F32, kind="ExternalInput")
    lab_loc = nc.dram_tensor("lab_loc", [R, 1], F32, kind="ExternalInput")
    tgt_loc = nc.dram_tensor("tgt_loc", [R, 1], F32, kind="ExternalInput")
    pfi = nc.dram_tensor("pfi", [KP, D], F32, kind="ExternalInput")
    pfp = nc.dram_tensor("pfp", [KP, D], F32, kind="ExternalInput")
    pidx = nc.dram_tensor("pidx", [128, nt_p * 8], mybir.dt.int16,
                          kind="ExternalInput")
    pvalid = nc.dram_tensor("pvalid", [128, nt_p], F32, kind="ExternalInput")
    acc_out = nc.dram_tensor("acc_out", [128, NCOL], F32, kind="ExternalOutput")

    def bcast_ap(handle, n):
        a = handle.ap() if isinstance(handle, bass.DRamTensorHandle) \
            else handle[:, :]
        return bass.AP(tensor=a.tensor, offset=a.offset, ap=[[0, 128], [1, n]])

    with tile.TileContext(nc) as tc:
        with (
            tc.tile_pool(name="persist", bufs=1) as persist,
            tc.tile_pool(name="stream", bufs=3) as stream,
            tc.tile_pool(name="dense", bufs=2) as dense,
            tc.tile_pool(name="small", bufs=2) as small,
            tc.tile_pool(name="gpsum", bufs=2, space="PSUM") as gpsum,
            tc.tile_pool(name="dscratch", bufs=1, space="DRAM") as dscratch,
        ):
            dma = nc.sync

            # DRAM scratch as pool tiles so Tile tracks store->read deps
            dp_dram = dscratch.tile([R, B], F32, tag="dp")
            rrow_dram = dscratch.tile([16, 128], F32, tag="rrow")
            srow_dram = dscratch.tile([16, 128], F32, tag="srow")

            # ---------- constants / setup ----------
            zeros = persist.tile([128, B], F32)
            nc.gpsimd.memset(zeros, 0.0)
            acc = persist.tile([128, NCOL], F32)
            nc.vector.memset(acc, 0.0)
            iota_c = persist.tile([128, C], F32)
            nc.gpsimd.iota(iota_c, pattern=[[1, C]], base=0,
                           channel_multiplier=0,
                           allow_small_or_imprecise_dtypes=True)
            # identity for PE transpose
            iota_sq = persist.tile([128, 128], F32)
            nc.gpsimd.iota(iota_sq, pattern=[[1, 128]], base=0,
                           channel_multiplier=0,
                           allow_small_or_imprecise_dtypes=True)
            pid = persist.tile([128, 1], F32)
            nc.gpsimd.iota(pid, pattern=[[0, 1]], base=0,
                           channel_multiplier=1,
                           allow_small_or_imprecise_dtypes=True)
            ident = persist.tile([128, 128], F32)
            nc.vector.tensor_scalar(out=ident, in0=iota_sq, scalar1=pid,
                                    scalar2=None, op0=ALU.is_equal)

            # ---------- big persistent loads ----------
            ft = []          # featT tiles [128, B] x KT
            dma_ring = [nc.sync, nc.scalar, nc.sync, nc.scalar]
            for k in range(KT):
                t = persist.tile([128, B], mybir.dt.bfloat16, tag=f"ft{k}")
                dma_ring[k % 4].dma_start(
                    out=t, in_=featT.ap()[k * 128:(k + 1) * 128, :])
                ft.append(t)
            ftl = []         # featT local tiles [128, R] x KT
            for k in range(KT):
                t = persist.tile([128, R], mybir.dt.bfloat16, tag=f"ftl{k}")
                dma.dma_start(out=t, in_=featTl.ap()[k * 128:(k + 1) * 128, :])
                ftl.append(t)

            lab_b = persist.tile([128, B], F32)
            nc.gpsimd.dma_start(out=lab_b, in_=bcast_ap(lab_all, B))

            # ---------- r_all via square+accum over natural feature tiles ----
            rcols = persist.tile([128, 16], F32)
            for t_i in range(16):
                fn_t = stream.tile([128, D], F32, tag="fn")
                (dma if t_i % 2 == 0 else nc.scalar).dma_start(
                    out=fn_t, in_=featN.ap()[t_i * 128:(t_i + 1) * 128, :])
                if t_i % 2 == 0:
                    nc.scalar.activation(out=fn_t, in_=fn_t, func=AF.Square,
                                         accum_out=rcols[:, t_i:t_i + 1])
                else:
                    nc.vector.scalar_tensor_tensor(
                        out=fn_t, in0=fn_t, scalar=1.0, in1=fn_t,
                        op0=ALU.mult, op1=ALU.mult,
                        accum_out=rcols[:, t_i:t_i + 1])

            rho_cols = persist.tile([128, 16], F32)
            nc.scalar.activation(out=rho_cols, in_=rcols, func=AF.Sqrt)
            s_cols = persist.tile([128, 16], F32)
            nc.vector.reciprocal(out=s_cols, in_=rho_cols)

            # transpose rcols/s_cols -> [16,128] rows -> DRAM -> broadcast
            for src, dst_dram in ((rcols, rrow_dram), (s_cols, srow_dram)):
                ps = gpsum.tile([16, 128], F32, tag="gpsum_big")
                nc.tensor.transpose(ps, src, ident)
                row_sb = small.tile([16, 128], F32, tag="rowsb")
                nc.scalar.copy(out=row_sb, in_=ps)
                dma.dma_start(out=dst_dram[:], in_=row_sb)

            r_b = persist.tile([128, B], F32)
            nc.gpsimd.dma_start(out=r_b, in_=bcast_ap(rrow_dram, B))
            s_b = persist.tile([128, B], F32)
            nc.gpsimd.dma_start(out=s_b, in_=bcast_ap(srow_dram, B))

            # ---------- local row stats ----------
            rloc = persist.tile([128, RT], F32)
            for m in range(RT):
                fl_t = stream.tile([128, D], F32, tag="fn")
                dma.dma_start(out=fl_t,
                              in_=featl.ap()[m * 128:(m + 1) * 128, :])
                nc.scalar.activation(out=fl_t, in_=fl_t, func=AF.Square,
                                     accum_out=rloc[:, m:m + 1])
            rho_loc = persist.tile([128, RT], F32)
            nc.scalar.activation(out=rho_loc, in_=rloc, func=AF.Sqrt)
            s_loc = persist.tile([128, RT], F32)
            nc.vector.reciprocal(out=s_loc, in_=rho_loc)

            lab_l = persist.tile([128, RT], F32)
            dma.dma_start(out=lab_l[:, 0:1], in_=lab_loc.ap()[0:128, :])
            dma.dma_start(out=lab_l[:, 1:2], in_=lab_loc.ap()[128:256, :])

            # ---------- focal + label smoothing ----------
            for m in range(RT):
                pred_t = stream.tile([128, C], F32, tag="pred")
                nc.scalar.dma_start(out=pred_t,
                                    in_=predl.ap()[m * 128:(m + 1) * 128, :])
                tgt_t = small.tile([128, 1], F32, tag="tgt")
                dma.dma_start(out=tgt_t,
                              in_=tgt_loc.ap()[m * 128:(m + 1) * 128, :])
                # sum(exp(pred)) -> lse (no max-subtraction; pred ~ randn)
                escr = stream.tile([128, C], F32, tag="escr")
                se = small.tile([128, 1], F32, tag="se")
                nc.scalar.activation(out=escr, in_=pred_t, func=AF.Exp,
                                     accum_out=se)
                lse = small.tile([128, 1], F32, tag="lse")
                nc.scalar.activation(out=lse, in_=se, func=AF.Ln)
                # sum(pred)
                spred = small.tile([128, 1], F32, tag="spred")
                nc.vector.scalar_tensor_tensor(
                    out=escr, in0=pred_t, scalar=1.0, in1=zeros[:, 0:C],
                    op0=ALU.mult, op1=ALU.add, accum_out=spred)
                # pred[target]
                tmask = stream.tile([128, C], F32, tag="tmask")
                nc.vector.tensor_scalar(out=tmask, in0=iota_c, scalar1=tgt_t,
                                        scalar2=None, op0=ALU.is_equal)
                ptgt = small.tile([128, 1], F32, tag="ptgt")
                nc.vector.scalar_tensor_tensor(
                    out=escr, in0=pred_t, scalar=1.0, in1=tmask,
                    op0=ALU.mult, op1=ALU.mult, accum_out=ptgt)
                # ce = lse - ptgt ; pt = exp(-ce)
                ce = small.tile([128, 1], F32, tag="ce")
                nc.vector.tensor_sub(ce, lse, ptgt)
                pt = small.tile([128, 1], F32, tag="pt")
                nc.scalar.activation(out=pt, in_=ce, func=AF.Exp, scale=-1.0)
                onept = small.tile([128, 1], F32, tag="onept")
                nc.vector.tensor_scalar(out=onept, in0=pt, scalar1=-1.0,
                                        scalar2=1.0, op0=ALU.mult, op1=ALU.add)
                f2 = small.tile([128, 1], F32, tag="f2")
                nc.vector.tensor_mul(f2, onept, onept)
                nc.vector.tensor_mul(
                    acc[:, COL_FOC + m:COL_FOC + m + 1], f2, ce)
                # ls_i = lse - OFF*spred - (0.9-OFF)*ptgt
                t1 = small.tile([128, 1], F32, tag="lst1")
                nc.vector.tensor_scalar(out=t1, in0=spred,
                                        scalar1=float(-OFF), scalar2=None,
                                        op0=ALU.mult)
                t2 = small.tile([128, 1], F32, tag="lst2")
                nc.vector.scalar_tensor_tensor(
                    out=t2, in0=ptgt,
                    scalar=float(-(1.0 - SMOOTHING - OFF)), in1=t1,
                    op0=ALU.mult, op1=ALU.add)
                nc.vector.tensor_add(
                    acc[:, COL_LS + m:COL_LS + m + 1], lse, t2)

            # ---------- pair smalls: ri, rp, G over pair features ----------
            pk = persist.tile([128, 3 * nt_p], F32)   # [d2 | ri | rp] columns
            pk_g = persist.tile([128, nt_p], F32)
            for g in range(nt_p):
                fi_t = stream.tile([128, D], F32, tag="pfi")
                nc.sync.dma_start(out=fi_t,
                                   in_=pfi.ap()[g * 128:(g + 1) * 128, :])
                fp_t = stream.tile([128, D], F32, tag="pfp")
                nc.scalar.dma_start(out=fp_t,
                                    in_=pfp.ap()[g * 128:(g + 1) * 128, :])
                scr_c = stream.tile([128, D], F32, tag="pscr")
                nc.vector.scalar_tensor_tensor(
                    out=scr_c, in0=fi_t, scalar=1.0, in1=fp_t,
                    op0=ALU.mult, op1=ALU.mult,
                    accum_out=pk_g[:, g:g + 1])
                nc.vector.scalar_tensor_tensor(
                    out=fi_t, in0=fi_t, scalar=1.0, in1=fi_t,
                    op0=ALU.mult, op1=ALU.mult,
                    accum_out=pk[:, nt_p + g:nt_p + g + 1])
                nc.vector.scalar_tensor_tensor(
                    out=fp_t, in0=fp_t, scalar=1.0, in1=fp_t,
                    op0=ALU.mult, op1=ALU.mult,
                    accum_out=pk[:, 2 * nt_p + g:2 * nt_p + g + 1])

            # d2 = ri + rp - 2*G  (into pk[:, 0:nt_p])
            t_neg2g = small.tile([128, nt_p], F32, tag="pneg2g")
            nc.vector.scalar_tensor_tensor(
                out=t_neg2g, in0=pk_g, scalar=-2.0,
                in1=pk[:, nt_p:2 * nt_p], op0=ALU.mult, op1=ALU.add)
            nc.vector.tensor_add(pk[:, 0:nt_p], t_neg2g,
                                 pk[:, 2 * nt_p:3 * nt_p])
            # roots of [d2 | ri | rp] in one ACT pass
            proots = persist.tile([128, 3 * nt_p], F32)
            nc.scalar.activation(out=proots, in_=pk, func=AF.Sqrt)
            # x = d + margin
            px = persist.tile([128, nt_p], F32)
            nc.vector.tensor_scalar(out=px, in0=proots[:, 0:nt_p],
                                    scalar1=float(T_MARGIN), scalar2=None,
                                    op0=ALU.add)
            # 1/sqrt(ri), 1/sqrt(rp)
            pinv = small.tile([128, 2 * nt_p], F32, tag="pinv")
            nc.vector.reciprocal(out=pinv, in_=proots[:, nt_p:3 * nt_p])
            # simp = G * inv_i * inv_p
            simp = small.tile([128, nt_p], F32, tag="simp")
            nc.vector.tensor_mul(simp, pk_g, pinv[:, 0:nt_p])
            nc.vector.tensor_mul(simp, simp, pinv[:, nt_p:2 * nt_p])

            pval_t = persist.tile([128, nt_p], F32)
            dma.dma_start(out=pval_t, in_=pvalid.ap())

            # pos pair loss: -ln(exp(simp/T) + 1e-8), masked, accumulated
            pexp = small.tile([128, nt_p], F32, tag="pexp")
            nc.scalar.activation(out=pexp, in_=simp, func=AF.Exp,
                                 scale=float(1.0 / TEMPERATURE))
            pexp1 = small.tile([128, nt_p], F32, tag="pexp1")
            nc.vector.tensor_scalar(out=pexp1, in0=pexp, scalar1=1e-8,
                                    scalar2=None, op0=ALU.add)
            pln = small.tile([128, nt_p], F32, tag="pln")
            nc.scalar.activation(out=pln, in_=pexp1, func=AF.Ln)
            nc.vector.scalar_tensor_tensor(
                out=pln, in0=pln, scalar=-1.0, in1=pval_t,
                op0=ALU.mult, op1=ALU.mult,
                accum_out=acc[:, COL_POS:COL_POS + 1])
            # neg correction: min(simp, 0.5) * valid
            pmc = small.tile([128, nt_p], F32, tag="pmc")
            nc.vector.tensor_scalar(out=pmc, in0=simp, scalar1=0.5,
                                    scalar2=None, op0=ALU.min)
            nc.vector.scalar_tensor_tensor(
                out=pmc, in0=pmc, scalar=1.0, in1=pval_t,
                op0=ALU.mult, op1=ALU.mult,
                accum_out=acc[:, COL_NCO:COL_NCO + 1])

            # ---------- dense phase: G matmul + sim/dist ----------
            for m in range(RT):
                gps = gpsum.tile([128, B], F32, tag="gpsum_big")
                for nchunk in range(NCHUNK):
                    for k in range(KT):
                        nc.tensor.matmul(
                            gps[:, nchunk * 512:(nchunk + 1) * 512],
                            ftl[k][:, m * 128:(m + 1) * 128],
                            ft[k][:, nchunk * 512:(nchunk + 1) * 512],
                            start=(k == 0), stop=(k == KT - 1),
                        )
                # lmb = (lab_b == lab_i) * BIGMASK
                lmb = dense.tile([128, B], F32, tag="dwork")
                nc.vector.tensor_scalar(out=lmb, in0=lab_b,
                                        scalar1=lab_l[:, m:m + 1],
                                        scalar2=BIGMASK,
                                        op0=ALU.is_equal, op1=ALU.mult)
                # rbl = lmb + r_i + r_j
                rbl = dense.tile([128, B], F32, tag="dwork")
                nc.vector.scalar_tensor_tensor(
                    out=rbl, in0=lmb, scalar=rloc[:, m:m + 1], in1=r_b,
                    op0=ALU.add, op1=ALU.add)
                # d2m = -2*G + rbl
                d2m = dense.tile([128, B], F32, tag="dwork")
                nc.vector.scalar_tensor_tensor(
                    out=d2m, in0=gps, scalar=-2.0, in1=rbl,
                    op0=ALU.mult, op1=ALU.add)
                # sim = (G * s_i) * s_j   -- also accumulate neg term
                sim = dense.tile([128, B], F32, tag="dwork")
                nc.vector.scalar_tensor_tensor(
                    out=sim, in0=gps, scalar=s_loc[:, m:m + 1], in1=s_b,
                    op0=ALU.mult, op1=ALU.mult)
                nc.vector.scalar_tensor_tensor(
                    out=sim, in0=sim, scalar=0.5, in1=zeros,
                    op0=ALU.subtract, op1=ALU.min,
                    accum_out=acc[:, COL_NEG + m:COL_NEG + m + 1])
                # D' = sqrt(relu(d2m))
                d2r = dense.tile([128, B], F32, tag="dwork")
                nc.scalar.activation(out=d2r, in_=d2m, func=AF.Relu)
                dpt = dense.tile([128, B], F32, tag="dwork")
                nc.scalar.activation(out=dpt, in_=d2r, func=AF.Sqrt)
                nc.scalar.dma_start(out=dp_dram[m * 128:(m + 1) * 128, :],
                                     in_=dpt)
                # self-pair triplet term: sum min(D' - margin, 0)
                nc.vector.scalar_tensor_tensor(
                    out=dpt, in0=dpt, scalar=float(T_MARGIN), in1=zeros,
                    op0=ALU.subtract, op1=ALU.min,
                    accum_out=acc[:, COL_SELF + m:COL_SELF + m + 1])

            # ---------- pair row gather + triplet accumulation ----------
            idx_sb = persist.tile([128, nt_p * 8], mybir.dt.int16)
            dma.dma_start(out=idx_sb, in_=pidx.ap())
            for g in range(nt_p):
                grow = stream.tile([128, 1, B], F32, tag="grow")
                nc.gpsimd.dma_gather(
                    out_ap=grow,
                    in_ap=dp_dram[:, :],
                    idxs_ap=idx_sb[:, g * 8:(g + 1) * 8],
                    num_idxs=128,
                    num_idxs_reg=128,
                    elem_size=B,
                )
                tmp_t = small.tile([128, 1], F32, tag="ptrip")
                nc.vector.scalar_tensor_tensor(
                    out=grow[:, 0, :], in0=grow[:, 0, :], scalar=px[:, g:g + 1],
                    in1=zeros, op0=ALU.subtract, op1=ALU.min,
                    accum_out=tmp_t)
                nc.vector.tensor_mul(
                    acc[:, COL_PAIR + g:COL_PAIR + g + 1], tmp_t,
                    pval_t[:, g:g + 1])

            # ---------- writeback ----------
            dma.dma_start(out=acc_out.ap(), in_=acc)

    nc.compile()
    meta = dict(nt_p=nt_p, NCOL=NCOL, COL_NEG=COL_NEG, COL_SELF=COL_SELF,
                COL_PAIR=COL_PAIR, COL_POS=COL_POS, COL_NCO=COL_NCO,
                COL_FOC=COL_FOC, COL_LS=COL_LS)
    _BUILD_CACHE[key] = (nc, meta)
    return nc, meta


def _host_prep(pred, target, features):
    """Build the 8 per-core input maps."""
    pred = np.asarray(pred, dtype=np.float32)
    target = np.asarray(target)
    features = np.asarray(features, dtype=np.float32)
    labels = target.astype(np.int64)

    featT = np.ascontiguousarray(features.T)               # [D, B]
    featT_bf = featT.astype(ml_dtypes.bfloat16)
    lab_f32 = labels.astype(np.float32)

    # same-label non-self pairs, grouped by anchor's core
    order = np.argsort(labels, kind="stable")
    sorted_lab = labels[order]
    # boundaries of equal-label runs
    starts = np.flatnonzero(np.r_[True, sorted_lab[1:] != sorted_lab[:-1]])
    ends = np.r_[starts[1:], len(sorted_lab)]
    pairs_i, pairs_p = [], []
    for s, e in zip(starts, ends):
        if e - s < 2:
            continue
        members = order[s:e]
        ii, pp = np.meshgrid(members, members, indexing="ij")
        m = ii != pp
        pairs_i.append(ii[m])
        pairs_p.append(pp[m])
    if pairs_i:
        pairs_i = np.concatenate(pairs_i)
        pairs_p = np.concatenate(pairs_p)
    else:
        pairs_i = np.zeros((0,), np.int64)
        pairs_p = np.zeros((0,), np.int64)
    k_real = len(pairs_i)

    core_of = pairs_i // R
    per_core = [(pairs_i[core_of == c], pairs_p[core_of == c])
                for c in range(N_CORES)]
    kmax = max((len(a) for a, _ in per_core), default=0)
    nt_p = max(1, math.ceil(kmax / 128))
    KP = nt_p * 128

    e1 = np.zeros((D,), np.float32)
    e1[0] = 1.0

    in_maps = []
    for c in range(N_CORES):
        pi, pp = per_core[c]
        k = len(pi)
        fi = np.empty((KP, D), np.float32)
        fp = np.empty((KP, D), np.float32)
        fi[:k] = features[pi]
        fp[:k] = features[pp]
        fi[k:] = e1
        fp[k:] = e1
        rowidx = np.zeros((KP,), np.int16)
        rowidx[:k] = (pi % R).astype(np.int16)
        valid = np.zeros((KP,), np.float32)
        valid[:k] = 1.0
        # gather idx layout: unwrapped[j] = idx_tile[j%16, j//16], per tile g
        # [p, g*8+s] = rowidx[g*128 + s*16 + p]; replicated into all 8
        # GPSIMD core windows (HW reads its own 16-partition group)
        idx16 = rowidx.reshape(nt_p, 8, 16).transpose(2, 0, 1).reshape(16, -1)
        pidx = np.ascontiguousarray(np.tile(idx16, (8, 1)))
        # valid layout: [128, nt_p] with [p, g] = valid[g*128+p]
        pval = np.ascontiguousarray(valid.reshape(nt_p, 128).T)

        rows = slice(c * R, (c + 1) * R)
        in_maps.append({
            "featT": featT_bf,
            "featN": features,
            "featTl": np.ascontiguousarray(featT_bf[:, rows]),
            "featl": np.ascontiguousarray(features[rows]),
            "predl": np.ascontiguousarray(pred[rows]),
            "lab_all": lab_f32,
            "lab_loc": np.ascontiguousarray(lab_f32[rows, None]),
            "tgt_loc": np.ascontiguousarray(lab_f32[rows, None]),
            "pfi": fi,
            "pfp": fp,
            "pidx": pidx,
            "pvalid": pval,
        })
    return in_maps, nt_p, k_real


def _combine(results, meta, k_real):
    """Host-side scalar all-reduce + final loss combination."""
    nt_p = meta["nt_p"]
    accs = np.stack([r["acc_out"] for r in results]).astype(np.float64)
    tot = accs.sum(axis=(0, 1))                 # [NCOL]

    neg_dense = -(tot[meta["COL_NEG"]] + tot[meta["COL_NEG"] + 1])
    self_trip = -(tot[meta["COL_SELF"]] + tot[meta["COL_SELF"] + 1])
    pair_trip = -tot[meta["COL_PAIR"]:meta["COL_PAIR"] + nt_p].sum()
    pair_pos = tot[meta["COL_POS"]]
    negcorr_min = tot[meta["COL_NCO"]]
    focal_sum = tot[meta["COL_FOC"]] + tot[meta["COL_FOC"] + 1]
    ls_sum = tot[meta["COL_LS"]] + tot[meta["COL_LS"] + 1]

    k_tot = k_real + B
    pos_self = B * (-np.log(np.exp(1.0 / TEMPERATURE) + 1e-8))
    pos_zero = (B * B - k_tot) * (-np.log1p(1e-8))
    pos_sum = pair_pos + pos_self + pos_zero
    neg_sum = neg_dense + negcorr_min + 0.5 * B

    lc = (pos_sum + neg_sum) / (B * B)
    lt = (self_trip + pair_trip) / (B + 1e-8)
    lf = ALPHA * focal_sum / B
    ls = ls_sum / B
    total = (W_CONTRASTIVE * lc + W_TRIPLET * lt
             + W_FOCAL * lf + W_LABEL_SMOOTH * ls)
    return np.array([lc, lt, lf, ls, total], dtype=np.float32)


def kernel(pred, target, features):
    in_maps, nt_p, k_real = _host_prep(pred, target, features)
    # fix the tgt entries (they must be the class targets, same as labels here)
    nc, meta = _build(nt_p)
    res = run_bass_kernel_spmd(nc, in_maps, core_ids=list(range(N_CORES)))
    return _combine(res.results, meta, k_real)


if __name__ == "__main__":
    import reference

    inputs = reference.setup_inputs()
    expected = np.asarray(reference.reference(**inputs))
    actual = kernel(**{k: np.asarray(v) for k, v in inputs.items()})
    err = np.abs(actual - expected) / np.maximum(np.abs(expected), 1e-12)
    print("expected:", expected)
    print("actual:  ", actual)
    print("rel err: ", err)
